# revision 53
# baseline (speedup 1.0000x reference)
"""DeepseekV2 MLA attention forward — Trainium2 Bass kernel (8 NeuronCores).

Sharding: data-parallel over batch (2) x sequence-parallel over query rows
(4 panels of 512) = 8 cores.  Each core:
  - computes the kv path (kv_a -> rmsnorm -> kv_b, k_pe rope) for its OWN
    512-row panel only (identical SPMD instructions on every core),
  - AllGathers (k_nope, V, roped k_pe) across the 4 cores of its batch,
    giving every core the full key set in panel-major (= original) order,
  - computes the q path for its panel, full attention over the gathered
    keys for all 16 heads, and o_proj.
Attention is key-permutation invariant, so gathering in rank order needs
no reordering.  The gather overlaps the q_a/q_b projections.

All matmul operands are bf16 (fp32 PSUM accumulation).  Weights are
pre-transposed on the host into per-partition-contiguous [m][p][k][c]
blocks so every weight DMA is one large contiguous burst.  Intermediates
stay resident in SBUF.  The attention kb loop is software-pipelined so
the PE never waits on the Act-engine exp; softmax normalization uses
reciprocal_approx_fast and is deferred one head so the PE never stalls.
"""

import os
import numpy as np
import ml_dtypes
from contextlib import ExitStack

import concourse.bass as bass
import concourse.bacc as bacc
import concourse.mybir as mybir
import concourse.tile as tile
from concourse import bass_utils

B, S, HID = 2, 2048, 2048
NH = 16
QLR, KVLR = 1536, 512
DN, DR, DV = 128, 64, 128
DQK = DN + DR
SCALE = DQK ** -0.5
EPS = 1e-6
P = 128
NPANEL = 4
W = S // NPANEL            # 512 query rows per core
NCORES = B * NPANEL

F32 = mybir.dt.float32
F32R = mybir.dt.float32r
BF16 = mybir.dt.bfloat16
NPBF = ml_dtypes.bfloat16
EXP = mybir.ActivationFunctionType.Exp
SQRT = mybir.ActivationFunctionType.Sqrt
SQUARE = mybir.ActivationFunctionType.Square
COPY = mybir.ActivationFunctionType.Copy
MULT = mybir.AluOpType.mult
ADD = mybir.AluOpType.add

KB_HID = HID // P          # 16
KB_QLR = QLR // P          # 12
KB_CKV = KVLR // P         # 4
KB_S = S // P              # 16
KB_W = W // P              # 4  key blocks in own panel
MB_QLR = QLR // P          # 12
MB_KVA = 5                 # 4 ckv blocks + 1 (zero-padded) rope block
MB_NOPE = NH * DN // P     # 16
MB_PE = NH * DR // P       # 8
MB_HID = HID // P          # 16

# gathered kv payload layout (bf16 elements per partition)
KN_ELE = NH * W            # 8192  k_nope  [h][key]
V_ELE = (NH // 2) * KB_W * 2 * DV   # 8192  V  [g][kb_own][2*dv]
KPE_ELE = W                # 512   roped k_pe (2x64 duplicated rows)
KV_ELE = KN_ELE + V_ELE + KPE_ELE   # 16896
V_OFF = KN_ELE
KPE_OFF = KN_ELE + V_ELE

LAST_RESULT = None         # BassKernelResults of the most recent launch


def _emit(tc, t, with_mask):
    nc = tc.nc
    mm = nc.tensor.matmul
    top = ExitStack()

    const = top.enter_context(tc.tile_pool(name="const", bufs=1))
    ones_col = const.tile([P, 1], BF16)
    nc.vector.memset(ones_col[:], 1.0)
    ones_row = const.tile([1, P], BF16)
    nc.vector.memset(ones_row[:], 1.0)
    eps1 = const.tile([1, 1], F32)
    nc.vector.memset(eps1[:], EPS)
    qa_ln = const.tile([P, MB_QLR], F32)
    nc.sync.dma_start(qa_ln[:], t["qa_ln_p"][:])
    kva_ln = const.tile([P, KB_CKV], F32)
    nc.sync.dma_start(kva_ln[:], t["kva_ln_p"][:])
    cos2p = const.tile([P, W], BF16)
    nc.sync.dma_start(cos2p[:], t["cos2p"][:])
    sin2sp = const.tile([P, W], BF16)
    nc.sync.dma_start(sin2sp[:], t["sin2sp"][:])

    # persistent SBUF intermediates (all bf16)
    persist = top.enter_context(tc.tile_pool(name="persist", bufs=1))
    qaT = persist.tile([P, MB_QLR, W], BF16)       # q_a output, normalized
    kpe2 = persist.tile([P, S], BF16)              # gathered roped k_pe
    qnopeT = persist.tile([P, MB_NOPE, W], BF16)
    qpeT = persist.tile([P, MB_PE, W], BF16)       # roped q_pe
    oT = persist.tile([P, NH, W], BF16)            # attn out (pre-o_proj)

    # Weight pools for later phases sit BELOW the per-phase scratch pools
    # in the SBUF stack, so their prefetch DMAs never carry a write-after-
    # read hazard against the previous phase's scratch tiles.
    paw = top.enter_context(tc.tile_pool(name="phA_w", bufs=2))
    pcw = top.enter_context(tc.tile_pool(name="phC_w", bufs=2))
    pdv = top.enter_context(tc.tile_pool(name="phD_v", bufs=2))
    pdk = top.enter_context(tc.tile_pool(name="phD_k", bufs=2))
    pew = top.enter_context(tc.tile_pool(name="phE_w", bufs=2))

    # own-panel hidden states + kv weights, prefetched first
    ph = top.enter_context(tc.tile_pool(name="hpanel", bufs=1))
    hn0 = ph.tile([P, KB_HID, W], BF16)
    nc.sync.dma_start(hn0[:], t["hs_pkp"][:])
    pbw = top.enter_context(tc.tile_pool(name="phB_w", bufs=1))
    wkva = pbw.tile([P, MB_KVA, KB_HID, P], BF16)
    nc.sync.dma_start(wkva[:], t["w_kva"][:])
    pkw = top.enter_context(tc.tile_pool(name="kvb_w", bufs=2))
    dram = top.enter_context(tc.tile_pool(name="dram", bufs=1, space="DRAM"))
    kv_out = dram.tile([P, KV_ELE], BF16)
    kv_gath = dram.tile([NPANEL, P, KV_ELE], BF16)

    def rsqrt_bcast(pool, psum_pool, ss_ps, inv_dim):
        """[1,n] sum-of-squares psum -> [P,n] f32 PSUM of 1/sqrt(mean+eps)."""
        n = ss_ps.shape[-1]
        srow = pool.tile([1, n], F32, tag="srow")
        nc.scalar.activation(srow[:], ss_ps[:], SQRT, bias=eps1[:],
                             scale=inv_dim)
        rrow = pool.tile([1, n], F32, tag="rrow")
        nc.vector.reciprocal_approx_fast(rrow[:], srow[:])
        rrow_bf = pool.tile([1, n], BF16, tag="rrow_bf")
        nc.scalar.activation(rrow_bf[:], rrow[:], COPY)
        bc_ps = psum_pool.tile([P, n], F32, tag="bcast")
        mm(bc_ps[:], ones_row[:], rrow_bf[:], start=True, stop=True)
        return bc_ps

    # ------------- phase B: own-panel kv_a + rmsnorm + kpe rope ------
    with tc.tile_pool(name="phB", bufs=2) as pb, \
         tc.tile_pool(name="kvbuf", bufs=1) as pkv, \
         tc.tile_pool(name="psA", bufs=3, space="PSUM") as psA, \
         tc.tile_pool(name="psS", bufs=1, space="PSUM") as psSS, \
         tc.tile_pool(name="psB", bufs=1, space="PSUM") as psBC:
        ckT = pkv.tile([P, KB_CKV, W], BF16)       # own compressed kv
        kno = pkv.tile([P, NH, W], BF16)           # own k_nope [h][key]
        von = pkv.tile([P, NH // 2, KB_W, 2 * DV], BF16)   # own V
        kpo = pkv.tile([P, W], BF16)               # own roped k_pe

        ss = psSS.tile([1, W], F32, tag="ss")
        kp = pb.tile([P, W], BF16, tag="kp")
        sq_prev = None
        for m in range(MB_KVA):
            ps = psA.tile([P, W], F32, tag="psA")
            for k in range(KB_HID):
                mm(ps[:], wkva[:, m, k, :], hn0[:, k, :],
                   start=(k == 0), stop=(k == KB_HID - 1))
            if m < KB_CKV:
                nc.scalar.activation(ckT[:, m, :], ps[:], COPY)
                sq = pb.tile([P, W], BF16, tag="sq")
                nc.scalar.activation(sq[:], ps[:], SQUARE)
                if sq_prev is not None:
                    mm(ss[:], ones_col[:], sq_prev,
                       start=(m == 1), stop=False, skip_group_check=True)
                sq_prev = sq[:]
            else:
                mm(ss[:], ones_col[:], sq_prev, start=False, stop=True,
                   skip_group_check=True)
                nc.scalar.activation(kp[0:DR, :], ps[0:DR, :], COPY)
                nc.vector.tensor_copy(kp[DR:P, :], ps[0:DR, :])
        rk = rsqrt_bcast(pb, psBC, ss[:], 1.0 / KVLR)
        for m in range(KB_CKV):
            nc.vector.scalar_tensor_tensor(
                ckT[:, m, :], ckT[:, m, :], kva_ln[:, m:m + 1], rk[:],
                MULT, MULT)
        # RoPE on kp (both 64-halves hold the same data)
        rot = pb.tile([P, W], BF16, tag="rot")
        for h in (0, DR):
            nc.vector.tensor_copy(rot[h:h + 32, :], kp[h + 32:h + 64, :])
            nc.vector.tensor_copy(rot[h + 32:h + 64, :], kp[h:h + 32, :])
        nc.vector.tensor_tensor(kp[:], kp[:], cos2p[:], MULT)
        nc.vector.tensor_tensor(rot[:], rot[:], sin2sp[:], MULT)
        nc.vector.tensor_tensor(kpo[:], kp[:], rot[:], ADD)

        # ------------- phase B2: own-panel kv_b (k_nope + V) ---------
        with tc.tile_pool(name="psK", bufs=3, space="PSUM") as psK:
            for h in range(NH):
                wkn = pkw.tile([P, KB_CKV, DN], BF16, tag="wkn")
                nc.sync.dma_start(wkn[:], t["w_kvb_kn"][h])
                psk = psK.tile([P, W], F32, tag="psk")
                for kc in range(KB_CKV):
                    mm(psk[:], wkn[:, kc, :], ckT[:, kc, :],
                       start=(kc == 0), stop=(kc == KB_CKV - 1))
                nc.scalar.activation(kno[:, h, :], psk[:], COPY)
            for g in range(NH // 2):
                wv = pkw.tile([P, KB_CKV, 2 * DV], BF16, tag="wv")
                nc.sync.dma_start(wv[:], t["w_kvb_v"][g])
                for kb in range(KB_W):
                    psv = psK.tile([P, W], F32, tag="psk")
                    for kc in range(KB_CKV):
                        mm(psv[:, :2 * DV],
                           ckT[:, kc, kb * P:(kb + 1) * P], wv[:, kc, :],
                           start=(kc == 0), stop=(kc == KB_CKV - 1))
                    nc.scalar.activation(von[:, g, kb, :],
                                         psv[:, :2 * DV], COPY)

        # ship own kv payload and gather the batch group's full set
        nc.sync.dma_start(
            kv_out[:, 0:KN_ELE], kno[:].rearrange("p h w -> p (h w)"))
        nc.sync.dma_start(
            kv_out[:, V_OFF:KPE_OFF],
            von[:].rearrange("p g b c -> p (g b c)"))
        nc.sync.dma_start(kv_out[:, KPE_OFF:KV_ELE], kpo[:])
        nc.gpsimd.collective_compute(
            "AllGather", mybir.AluOpType.bypass,
            replica_groups=t["replica_groups"],
            ins=[kv_out.opt()], outs=[kv_gath.opt()])

    # ------------- phase A: qaT panel + rmsnorm ----------------------
    with tc.tile_pool(name="phA", bufs=2) as pa, \
         tc.tile_pool(name="psA", bufs=3, space="PSUM") as psA, \
         tc.tile_pool(name="psS", bufs=1, space="PSUM") as psSS, \
         tc.tile_pool(name="psB", bufs=1, space="PSUM") as psBC:
        ss = psSS.tile([1, W], F32, tag="ss")
        sq_prev = None
        for m in range(MB_QLR):
            wm = paw.tile([P, KB_HID, P], BF16, tag="wqa")
            nc.sync.dma_start(wm[:], t["w_qa"][m])
            ps = psA.tile([P, W], F32, tag="psA")
            for k in range(KB_HID):
                mm(ps[:], wm[:, k, :], hn0[:, k, :],
                   start=(k == 0), stop=(k == KB_HID - 1))
            nc.scalar.activation(qaT[:, m, :], ps[:], COPY)
            sq = pa.tile([P, W], BF16, tag="sq")
            nc.scalar.activation(sq[:], ps[:], SQUARE)
            if sq_prev is not None:
                mm(ss[:], ones_col[:], sq_prev,
                   start=(m == 1), stop=False, skip_group_check=True)
            sq_prev = sq[:]
        mm(ss[:], ones_col[:], sq_prev, start=False, stop=True,
           skip_group_check=True)
        rq = rsqrt_bcast(pa, psBC, ss[:], 1.0 / QLR)
        for m in range(MB_QLR):
            nc.vector.scalar_tensor_tensor(
                qaT[:, m, :], qaT[:, m, :], qa_ln[:, m:m + 1], rq[:],
                MULT, MULT)

    # ------------- phase C: q_b panel (+ RoPE on pe part) ------------
    with tc.tile_pool(name="phC", bufs=2) as pc, \
         tc.tile_pool(name="psA", bufs=3, space="PSUM") as psA:
        for m in range(MB_NOPE + MB_PE):
            wm = pcw.tile([P, KB_QLR, P], BF16, tag="wqb")
            nc.sync.dma_start(wm[:], t["w_qb"][m])
            ps = psA.tile([P, W], F32, tag="psA")
            for k in range(KB_QLR):
                mm(ps[:], wm[:, k, :], qaT[:, k, :],
                   start=(k == 0), stop=(k == KB_QLR - 1))
            if m < MB_NOPE:
                nc.scalar.activation(qnopeT[:, m, :], ps[:], COPY)
            else:
                j = m - MB_NOPE
                qb = pc.tile([P, W], BF16, tag="qb")
                nc.scalar.activation(qb[:], ps[:], COPY)
                rotq = pc.tile([P, W], BF16, tag="rotq")
                for h in (0, DR):
                    nc.vector.tensor_copy(rotq[h:h + 32, :],
                                          qb[h + 32:h + 64, :])
                    nc.vector.tensor_copy(rotq[h + 32:h + 64, :],
                                          qb[h:h + 32, :])
                nc.vector.tensor_tensor(rotq[:], rotq[:], sin2sp[:], MULT)
                nc.vector.tensor_tensor(qpeT[:, j, :], qb[:], cos2p[:], MULT)
                nc.vector.tensor_tensor(qpeT[:, j, :], qpeT[:, j, :],
                                        rotq[:], ADD)

    # ------------- phase D: attention over gathered keys -------------
    with tc.tile_pool(name="phD", bufs=2) as pd, \
         tc.tile_pool(name="probs", bufs=4) as pprob, \
         tc.tile_pool(name="psSc", bufs=4, space="PSUM") as psSc, \
         tc.tile_pool(name="psO", bufs=2, space="PSUM") as psO, \
         tc.tile_pool(name="psR", bufs=1, space="PSUM") as psR, \
         tc.tile_pool(name="psB2", bufs=1, space="PSUM") as psB2:
        dctx = ExitStack()
        mask_pool = None
        if with_mask:
            mask_pool = dctx.enter_context(tc.tile_pool(name="maskp", bufs=4))

        for r in range(NPANEL):
            nc.sync.dma_start(kpe2[:, r * W:(r + 1) * W],
                              kv_gath[r, :, KPE_OFF:KV_ELE])

        # deferred normalization finish of the previous head, emitted
        # late so its PE bcast / DVE reciprocal never stall the in-order
        # PE stream
        def finish_head(h, po, pr):
            rrow = pd.tile([1, W], F32, tag="rrow")
            nc.vector.reciprocal_approx_fast(rrow[:], pr[:])
            rrow_bf = pd.tile([1, W], BF16, tag="rrow_bf")
            nc.scalar.activation(rrow_bf[:], rrow[:], COPY)
            bc_ps = psB2.tile([P, W], F32, tag="bcd")
            mm(bc_ps[:], ones_row[:], rrow_bf[:], start=True, stop=True)
            bci = pd.tile([P, W], F32, tag="bci")
            nc.scalar.activation(bci[:], bc_ps[:], COPY)
            nc.vector.tensor_tensor(oT[:, h, :], po[:], bci[:], MULT)

        pending = None
        for g in range(NH // 2):
            # V for the 2 heads of this group, gathered panel-major
            v_sb = pdv.tile([P, KB_S, 2 * DV], BF16, tag="v")
            for r in range(NPANEL):
                nc.sync.dma_start(
                    v_sb[:, r * KB_W:(r + 1) * KB_W, :],
                    kv_gath[r, :, V_OFF + g * KB_W * 2 * DV:
                            V_OFF + (g + 1) * KB_W * 2 * DV])

            for hl in range(2):
                h = g * 2 + hl
                knT = pdk.tile([P, KB_S, P], BF16, tag="knT")
                for r in range(NPANEL):
                    nc.sync.dma_start(
                        knT[:, r * KB_W:(r + 1) * KB_W, :],
                        kv_gath[r, :, h * W:(h + 1) * W])

                # attention for head h, software-pipelined over kb
                po = psO.tile([P, W], F32, tag="po")
                pr = psR.tile([1, W], F32, tag="pr")
                hp64 = hl * DR
                probs_q = []
                for kb in range(KB_S):
                    pss = psSc.tile([P, W], F32, tag="pss")
                    mm(pss[:], knT[:, kb, :], qnopeT[:, h, :],
                       start=True, stop=False)
                    mm(pss[:], kpe2[hp64:hp64 + DR, kb * P:(kb + 1) * P],
                       qpeT[hp64:hp64 + DR, g, :], start=False, stop=True)
                    probs = pprob.tile([P, W], BF16, tag="probs")
                    if with_mask:
                        mtile = mask_pool.tile([P, W], F32, tag="mt")
                        nc.sync.dma_start(
                            mtile[:], t["maskT"][kb * P:(kb + 1) * P, :])
                        pf = pprob.tile([P, W], F32, tag="probs_f")
                        nc.vector.scalar_tensor_tensor(
                            pf[:], pss[:], SCALE, mtile[:], MULT, ADD)
                        nc.scalar.activation(probs[:], pf[:], EXP)
                    else:
                        nc.scalar.activation(probs[:], pss[:], EXP,
                                             scale=SCALE)
                    probs_q.append((kb, probs))
                    if len(probs_q) == 4 or kb == KB_S - 1:
                        for kb2, pb2 in probs_q:
                            mm(po[:], v_sb[:, kb2, hl * DV:(hl + 1) * DV],
                               pb2[:], start=(kb2 == 0),
                               stop=(kb2 == KB_S - 1), skip_group_check=True)
                            mm(pr[:], ones_col[:], pb2[:],
                               start=(kb2 == 0), stop=(kb2 == KB_S - 1),
                               skip_group_check=True)
                        probs_q = []
                        # finish the previous head once this head's first
                        # kb batch is in flight: its reciprocal has had
                        # time to drain, so the PE bcast never stalls
                        if pending is not None:
                            finish_head(*pending)
                            pending = None
                pending = (h, po, pr)
        finish_head(*pending)
        dctx.close()

    # ------------- phase E: o_proj -----------------------------------
    with tc.tile_pool(name="phE", bufs=2) as pe, \
         tc.tile_pool(name="psA", bufs=3, space="PSUM") as psA:
        for m in range(MB_HID):
            wm = pew.tile([P, NH, P], BF16, tag="wo")
            nc.sync.dma_start(wm[:], t["w_o"][m])
            ps = psA.tile([P, W], F32, tag="psA")
            for k in range(NH):
                mm(ps[:], wm[:, k, :], oT[:, k, :],
                   start=(k == 0), stop=(k == NH - 1))
            osb = pe.tile([P, W], F32, tag="osb")
            nc.scalar.activation(osb[:], ps[:], COPY)
            nc.sync.dma_start(t["outT"][m * P:(m + 1) * P, :], osb[:])
    top.close()


def _build_program(with_mask):
    nc = bacc.Bacc("TRN2", target_bir_lowering=False, debug=False)
    t = {"replica_groups": [list(range(b * NPANEL, (b + 1) * NPANEL))
                            for b in range(B)]}

    def inp(name, shape, dt=BF16):
        t[name] = nc.dram_tensor(name, list(shape), dt,
                                 kind="ExternalInput").ap()

    inp("hs_pkp", [P, KB_HID, W])
    inp("w_qa", [MB_QLR, P, KB_HID, P])
    inp("w_qb", [MB_NOPE + MB_PE, P, KB_QLR, P])
    inp("w_kva", [P, MB_KVA, KB_HID, P])
    inp("w_kvb_kn", [NH, P, KB_CKV, DN])
    inp("w_kvb_v", [NH // 2, P, KB_CKV, 2 * DV])
    inp("w_o", [MB_HID, P, NH, P])
    inp("qa_ln_p", [P, MB_QLR], F32)
    inp("kva_ln_p", [P, KB_CKV], F32)
    inp("cos2p", [P, W])
    inp("sin2sp", [P, W])
    if with_mask:
        inp("maskT", [S, W], F32)
    t["outT"] = nc.dram_tensor("outT", [HID, W], F32,
                               kind="ExternalOutput").ap()

    with tile.TileContext(nc) as tc:
        _emit(tc, t, with_mask)
    nc.compile()
    return nc


_PROG_CACHE = {}


def _get_program(with_mask):
    if with_mask not in _PROG_CACHE:
        _PROG_CACHE[with_mask] = _build_program(with_mask)
    return _PROG_CACHE[with_mask]


def _block4(w, mb, kb):
    """[kb*P, mb*P] -> [mb, P, kb, P] with W[m,p,k,c] = w[k*P+p, m*P+c]."""
    return np.ascontiguousarray(
        w.reshape(kb, P, mb, P).transpose(2, 1, 0, 3))


def make_in_maps(hidden_states, attention_mask, cos, sin, w_qa, qa_ln, w_qb,
                 w_kva, kva_ln, w_kvb, w_o, with_mask):
    f32 = np.float32
    c = np.ascontiguousarray

    w_qb_r = np.asarray(w_qb, f32).reshape(QLR, NH, DQK)
    w_qb_re = np.concatenate(
        [w_qb_r[:, :, :DN].reshape(QLR, NH * DN),
         w_qb_r[:, :, DN:].reshape(QLR, NH * DR)], axis=1)
    w_kva_pad = np.concatenate(
        [np.asarray(w_kva, f32), np.zeros((HID, P - DR), f32)], axis=1)
    kvb = np.asarray(w_kvb, f32).reshape(KB_CKV, P, NH, DN + DV)
    w_kvb_kn = c(kvb[:, :, :, :DN].transpose(2, 1, 0, 3)
                 .astype(NPBF))                        # [NH, P, KB_CKV, DN]
    w_kvb_v = c(kvb[:, :, :, DN:].reshape(KB_CKV, P, NH // 2, 2 * DV)
                .transpose(2, 1, 0, 3).astype(NPBF))   # [NH/2, P, kc, 256]

    qa_ln_p = c(np.asarray(qa_ln, f32).reshape(MB_QLR, P).T)
    kva_ln_p = c(np.asarray(kva_ln, f32).reshape(KB_CKV, P).T)

    cosT = np.asarray(cos, f32).T                      # [64, S]
    sinT = np.asarray(sin, f32).T
    sin_s = np.concatenate([-sinT[:DR // 2], sinT[DR // 2:]], axis=0)
    cos2 = c(np.concatenate([cosT, cosT], axis=0))     # [128, S]
    sin2s = c(np.concatenate([sin_s, sin_s], axis=0))

    shared = {
        "w_qa": _block4(np.asarray(w_qa, f32), MB_QLR, KB_HID).astype(NPBF),
        "w_qb": _block4(w_qb_re, MB_NOPE + MB_PE, KB_QLR).astype(NPBF),
        "w_kva": c(w_kva_pad.reshape(KB_HID, P, MB_KVA, P)
                   .transpose(1, 2, 0, 3).astype(NPBF)),
        "w_kvb_kn": w_kvb_kn,
        "w_kvb_v": w_kvb_v,
        "w_o": _block4(np.asarray(w_o, f32), MB_HID, KB_HID).astype(NPBF),
        "qa_ln_p": qa_ln_p,
        "kva_ln_p": kva_ln_p,
    }

    hs = np.asarray(hidden_states)
    am = np.asarray(attention_mask)
    in_maps = []
    for core in range(NCORES):
        b, pnl = divmod(core, NPANEL)
        q0 = pnl * W
        hsT = np.asarray(hs[b], f32).T[:, q0:q0 + W]   # [HID, W]
        m = dict(shared)
        m["hs_pkp"] = c(hsT.reshape(KB_HID, P, W).transpose(1, 0, 2)
                        .astype(NPBF))                 # [128, 16, W]
        m["cos2p"] = c(cos2[:, q0:q0 + W].astype(NPBF))
        m["sin2sp"] = c(sin2s[:, q0:q0 + W].astype(NPBF))
        if with_mask:
            m["maskT"] = c(am[b, 0, q0:q0 + W, :].T.astype(f32))
        in_maps.append(m)
    return in_maps


def kernel(hidden_states, attention_mask, cos, sin, w_qa, qa_ln, w_qb,
           w_kva, kva_ln, w_kvb, w_o):
    global LAST_RESULT
    with_mask = bool(np.any(np.asarray(attention_mask) != 0))
    nc = _get_program(with_mask)
    in_maps = make_in_maps(hidden_states, attention_mask, cos, sin, w_qa,
                           qa_ln, w_qb, w_kva, kva_ln, w_kvb, w_o, with_mask)
    trace = os.environ.get("KERNEL_TRACE", "0") == "1"
    res = bass_utils.run_bass_kernel_spmd(
        nc, in_maps, core_ids=list(range(NCORES)), trace=trace)
    LAST_RESULT = res

    out = np.empty((B, S, HID), np.float32)
    for core in range(NCORES):
        b, pnl = divmod(core, NPANEL)
        q0 = pnl * W
        out[b, q0:q0 + W, :] = res.results[core]["outT"].T
    return out


# revision 61
# speedup vs baseline: 1.0747x; 1.0747x over previous
"""DeepseekV2 MLA attention forward — Trainium2 Bass kernel (8 NeuronCores).

Sharding: data-parallel over batch (2) x sequence-parallel over query rows
(4 panels of 512) = 8 cores.  Each core:
  - computes the kv path (kv_a -> rmsnorm -> kv_b, k_pe rope) for its OWN
    512-row panel only (identical SPMD instructions on every core),
  - AllGathers (k_nope, V, roped k_pe) across the 4 cores of its batch,
    giving every core the full key set in panel-major (= original) order,
  - computes the q path for its panel, full attention over the gathered
    keys for all 16 heads, and o_proj.
Attention is key-permutation invariant, so gathering in rank order needs
no reordering.  The gather overlaps the q_a/q_b projections.

All matmul operands are bf16 (fp32 PSUM accumulation).  Weights are
pre-transposed on the host into per-partition-contiguous [m][p][k][c]
blocks so every weight DMA is one large contiguous burst.  Intermediates
stay resident in SBUF.  The attention kb loop is software-pipelined so
the PE never waits on the Act-engine exp; softmax normalization uses
reciprocal_approx_fast and is deferred one head so the PE never stalls.
"""

import os
import numpy as np
import ml_dtypes
from contextlib import ExitStack

import concourse.bass as bass
import concourse.bacc as bacc
import concourse.mybir as mybir
import concourse.tile as tile
from concourse import bass_utils

B, S, HID = 2, 2048, 2048
NH = 16
QLR, KVLR = 1536, 512
DN, DR, DV = 128, 64, 128
DQK = DN + DR
SCALE = DQK ** -0.5
EPS = 1e-6
P = 128
NPANEL = 4
W = S // NPANEL            # 512 query rows per core
NCORES = B * NPANEL

F32 = mybir.dt.float32
F32R = mybir.dt.float32r
BF16 = mybir.dt.bfloat16
NPBF = ml_dtypes.bfloat16
EXP = mybir.ActivationFunctionType.Exp
SQRT = mybir.ActivationFunctionType.Sqrt
SQUARE = mybir.ActivationFunctionType.Square
COPY = mybir.ActivationFunctionType.Copy
MULT = mybir.AluOpType.mult
ADD = mybir.AluOpType.add

KB_HID = HID // P          # 16
KB_QLR = QLR // P          # 12
KB_CKV = KVLR // P         # 4
KB_S = S // P              # 16
KB_W = W // P              # 4  key blocks in own panel
MB_QLR = QLR // P          # 12
MB_KVA = 5                 # 4 ckv blocks + 1 (zero-padded) rope block
MB_NOPE = NH * DN // P     # 16
MB_PE = NH * DR // P       # 8
MB_HID = HID // P          # 16

# gathered kv payload layout (bf16 elements per partition)
KN_ELE = NH * W            # 8192  k_nope  [h][key]
V_ELE = (NH // 2) * KB_W * 2 * DV   # 8192  V  [g][kb_own][2*dv]
KPE_ELE = W                # 512   roped k_pe (2x64 duplicated rows)
KV_ELE = KN_ELE + V_ELE + KPE_ELE   # 16896
V_OFF = KN_ELE
KPE_OFF = KN_ELE + V_ELE

LAST_RESULT = None         # BassKernelResults of the most recent launch


def _emit(tc, t, with_mask):
    nc = tc.nc
    mm = nc.tensor.matmul
    top = ExitStack()

    const = top.enter_context(tc.tile_pool(name="const", bufs=1))
    ones_col = const.tile([P, 1], BF16)
    nc.vector.memset(ones_col[:], 1.0)
    ones_row = const.tile([1, P], BF16)
    nc.vector.memset(ones_row[:], 1.0)
    eps1 = const.tile([1, 1], F32)
    nc.vector.memset(eps1[:], EPS)
    qa_ln = const.tile([P, MB_QLR], F32)
    nc.sync.dma_start(qa_ln[:], t["qa_ln_p"][:])
    kva_ln = const.tile([P, KB_CKV], F32)
    nc.sync.dma_start(kva_ln[:], t["kva_ln_p"][:])
    cos2p = const.tile([P, W], BF16)
    nc.sync.dma_start(cos2p[:], t["cos2p"][:])
    sin2sp = const.tile([P, W], BF16)
    nc.sync.dma_start(sin2sp[:], t["sin2sp"][:])

    # persistent SBUF intermediates (all bf16)
    persist = top.enter_context(tc.tile_pool(name="persist", bufs=1))
    qaT = persist.tile([P, MB_QLR, W], BF16)       # q_a output, normalized
    kpe2 = persist.tile([P, S], BF16)              # gathered roped k_pe
    qnopeT = persist.tile([P, MB_NOPE, W], BF16)
    qpeT = persist.tile([P, MB_PE, W], BF16)       # roped q_pe
    oT = persist.tile([P, NH, W], BF16)            # attn out (pre-o_proj)

    # Weight pools for later phases sit BELOW the per-phase scratch pools
    # in the SBUF stack, so their prefetch DMAs never carry a write-after-
    # read hazard against the previous phase's scratch tiles.  w_qb is
    # preloaded whole so phase C has no DMA dependence at all once the
    # collective starts competing for DMA queues.
    pqb = top.enter_context(tc.tile_pool(name="wqb_all", bufs=1))
    wqb = pqb.tile([P, MB_NOPE + MB_PE, KB_QLR, P], BF16)

    # own-panel hidden states, prefetched first
    ph = top.enter_context(tc.tile_pool(name="hpanel", bufs=1))
    hn0 = ph.tile([P, KB_HID, W], BF16)
    nc.sync.dma_start(hn0[:], t["hs_pkp"][:])
    paw = top.enter_context(tc.tile_pool(name="phA_w", bufs=3))
    pkw = top.enter_context(tc.tile_pool(name="kvb_w", bufs=2))
    dram = top.enter_context(tc.tile_pool(name="dram", bufs=1, space="DRAM"))
    kv_out = dram.tile([P, KV_ELE], BF16)
    kv_gath = dram.tile([NPANEL, P, KV_ELE], BF16)

    def rsqrt_bcast(pool, psum_pool, ss_ps, inv_dim):
        """[1,n] sum-of-squares psum -> [P,n] f32 PSUM of 1/sqrt(mean+eps)."""
        n = ss_ps.shape[-1]
        srow = pool.tile([1, n], F32, tag="srow")
        nc.scalar.activation(srow[:], ss_ps[:], SQRT, bias=eps1[:],
                             scale=inv_dim)
        rrow = pool.tile([1, n], F32, tag="rrow")
        nc.vector.reciprocal_approx_fast(rrow[:], srow[:])
        rrow_bf = pool.tile([1, n], BF16, tag="rrow_bf")
        nc.scalar.activation(rrow_bf[:], rrow[:], COPY)
        bc_ps = psum_pool.tile([P, n], F32, tag="bcast")
        mm(bc_ps[:], ones_row[:], rrow_bf[:], start=True, stop=True)
        return bc_ps

    # ------------- phase B: own-panel kv_a + rmsnorm + kpe rope ------
    with tc.tile_pool(name="phB", bufs=2) as pb, \
         tc.tile_pool(name="phB_w", bufs=2) as pbw, \
         tc.tile_pool(name="kvbuf", bufs=1) as pkv, \
         tc.tile_pool(name="psA", bufs=3, space="PSUM") as psA, \
         tc.tile_pool(name="psS", bufs=1, space="PSUM") as psSS, \
         tc.tile_pool(name="psB", bufs=1, space="PSUM") as psBC:
        ckT = pkv.tile([P, KB_CKV, W], BF16)       # own compressed kv
        kpo = pkv.tile([P, W], BF16)               # own roped k_pe

        ss = psSS.tile([1, W], F32, tag="ss")
        kp = pb.tile([P, W], BF16, tag="kp")
        sq_prev = None
        for m in range(MB_KVA):
            wkva = pbw.tile([P, KB_HID, P], BF16, tag="wkva")
            nc.sync.dma_start(wkva[:], t["w_kva"][:, m])
            ps = psA.tile([P, W], F32, tag="psA")
            for k in range(KB_HID):
                mm(ps[:], wkva[:, k, :], hn0[:, k, :],
                   start=(k == 0), stop=(k == KB_HID - 1))
            if m < KB_CKV:
                nc.scalar.activation(ckT[:, m, :], ps[:], COPY)
                sq = pb.tile([P, W], BF16, tag="sq")
                nc.scalar.activation(sq[:], ps[:], SQUARE)
                if sq_prev is not None:
                    mm(ss[:], ones_col[:], sq_prev,
                       start=(m == 1), stop=False, skip_group_check=True)
                sq_prev = sq[:]
            else:
                mm(ss[:], ones_col[:], sq_prev, start=False, stop=True,
                   skip_group_check=True)
                nc.scalar.activation(kp[0:DR, :], ps[0:DR, :], COPY)
                nc.vector.tensor_copy(kp[DR:P, :], ps[0:DR, :])
        rk = rsqrt_bcast(pb, psBC, ss[:], 1.0 / KVLR)
        for m in range(KB_CKV):
            nc.vector.scalar_tensor_tensor(
                ckT[:, m, :], ckT[:, m, :], kva_ln[:, m:m + 1], rk[:],
                MULT, MULT)
        # RoPE on kp (both 64-halves hold the same data)
        rot = pb.tile([P, W], BF16, tag="rot")
        for h in (0, DR):
            nc.vector.tensor_copy(rot[h:h + 32, :], kp[h + 32:h + 64, :])
            nc.vector.tensor_copy(rot[h + 32:h + 64, :], kp[h:h + 32, :])
        nc.vector.tensor_tensor(kp[:], kp[:], cos2p[:], MULT)
        nc.vector.tensor_tensor(rot[:], rot[:], sin2sp[:], MULT)
        nc.vector.tensor_tensor(kpo[:], kp[:], rot[:], ADD)

        # ------------- phase B2: own-panel kv_b (k_nope + V) ---------
        # outputs stream straight to the DRAM collective bounce buffer
        with tc.tile_pool(name="psK", bufs=3, space="PSUM") as psK, \
             tc.tile_pool(name="kvstage", bufs=3) as pst:
            for h in range(NH):
                wkn = pkw.tile([P, KB_CKV, DN], BF16, tag="wkn")
                nc.sync.dma_start(wkn[:], t["w_kvb_kn"][h])
                psk = psK.tile([P, W], F32, tag="psk")
                for kc in range(KB_CKV):
                    mm(psk[:], wkn[:, kc, :], ckT[:, kc, :],
                       start=(kc == 0), stop=(kc == KB_CKV - 1))
                kst = pst.tile([P, W], BF16, tag="kst")
                nc.scalar.activation(kst[:], psk[:], COPY)
                nc.sync.dma_start(kv_out[:, h * W:(h + 1) * W], kst[:])
            for g in range(NH // 2):
                wv = pkw.tile([P, KB_CKV, 2 * DV], BF16, tag="wv")
                nc.sync.dma_start(wv[:], t["w_kvb_v"][g])
                for kb in range(KB_W):
                    psv = psK.tile([P, W], F32, tag="psk")
                    for kc in range(KB_CKV):
                        mm(psv[:, :2 * DV],
                           ckT[:, kc, kb * P:(kb + 1) * P], wv[:, kc, :],
                           start=(kc == 0), stop=(kc == KB_CKV - 1))
                    vst = pst.tile([P, 2 * DV], BF16, tag="vst")
                    nc.scalar.activation(vst[:], psv[:, :2 * DV], COPY)
                    nc.sync.dma_start(
                        kv_out[:, V_OFF + (g * KB_W + kb) * 2 * DV:
                               V_OFF + (g * KB_W + kb + 1) * 2 * DV],
                        vst[:])
            nc.sync.dma_start(kv_out[:, KPE_OFF:KV_ELE], kpo[:])

    # preload ALL of w_qb now: its descriptors sit ahead of the
    # collective's transfers in the DMA queues, so phase C never waits
    # behind the gather
    nc.sync.dma_start(wqb[:], t["w_qb"][:])

    # phase D DMA-destination pools (opened after B's scratch is freed,
    # before C's scratch, so D's gathered-kv DMAs carry no WAR hazard)
    pdv = top.enter_context(tc.tile_pool(name="phD_v", bufs=1))
    pdk = top.enter_context(tc.tile_pool(name="phD_k", bufs=2))
    pew = top.enter_context(tc.tile_pool(name="phE_w", bufs=2))

    # ------------- phase A: qaT panel + rmsnorm ----------------------
    with tc.tile_pool(name="phA", bufs=2) as pa, \
         tc.tile_pool(name="psA", bufs=3, space="PSUM") as psA, \
         tc.tile_pool(name="psS", bufs=1, space="PSUM") as psSS, \
         tc.tile_pool(name="psB", bufs=1, space="PSUM") as psBC:
        ss = psSS.tile([1, W], F32, tag="ss")
        sq_prev = None
        for m in range(MB_QLR):
            wm = paw.tile([P, KB_HID, P], BF16, tag="wqa")
            nc.sync.dma_start(wm[:], t["w_qa"][m])
            ps = psA.tile([P, W], F32, tag="psA")
            for k in range(KB_HID):
                mm(ps[:], wm[:, k, :], hn0[:, k, :],
                   start=(k == 0), stop=(k == KB_HID - 1))
            nc.scalar.activation(qaT[:, m, :], ps[:], COPY)
            sq = pa.tile([P, W], BF16, tag="sq")
            nc.scalar.activation(sq[:], ps[:], SQUARE)
            if sq_prev is not None:
                mm(ss[:], ones_col[:], sq_prev,
                   start=(m == 1), stop=False, skip_group_check=True)
            sq_prev = sq[:]
        mm(ss[:], ones_col[:], sq_prev, start=False, stop=True,
           skip_group_check=True)
        rq = rsqrt_bcast(pa, psBC, ss[:], 1.0 / QLR)
        for m in range(MB_QLR):
            nc.vector.scalar_tensor_tensor(
                qaT[:, m, :], qaT[:, m, :], qa_ln[:, m:m + 1], rq[:],
                MULT, MULT)

    # gather the batch group's full kv set; fires as soon as the B2
    # DMAs land, overlapping the q_a/q_b projections
    nc.gpsimd.collective_compute(
        "AllGather", mybir.AluOpType.bypass,
        replica_groups=t["replica_groups"],
        ins=[kv_out.opt()], outs=[kv_gath.opt()])

    # ------------- phase C: q_b panel (+ RoPE on pe part) ------------
    with tc.tile_pool(name="phC", bufs=2) as pc, \
         tc.tile_pool(name="psA", bufs=3, space="PSUM") as psA:
        for m in range(MB_NOPE + MB_PE):
            ps = psA.tile([P, W], F32, tag="psA")
            for k in range(KB_QLR):
                mm(ps[:], wqb[:, m, k, :], qaT[:, k, :],
                   start=(k == 0), stop=(k == KB_QLR - 1))
            if m < MB_NOPE:
                nc.scalar.activation(qnopeT[:, m, :], ps[:], COPY)
            else:
                j = m - MB_NOPE
                qb = pc.tile([P, W], BF16, tag="qb")
                nc.scalar.activation(qb[:], ps[:], COPY)
                rotq = pc.tile([P, W], BF16, tag="rotq")
                for h in (0, DR):
                    nc.vector.tensor_copy(rotq[h:h + 32, :],
                                          qb[h + 32:h + 64, :])
                    nc.vector.tensor_copy(rotq[h + 32:h + 64, :],
                                          qb[h:h + 32, :])
                nc.vector.tensor_tensor(rotq[:], rotq[:], sin2sp[:], MULT)
                nc.vector.tensor_tensor(qpeT[:, j, :], qb[:], cos2p[:], MULT)
                nc.vector.tensor_tensor(qpeT[:, j, :], qpeT[:, j, :],
                                        rotq[:], ADD)

    # ------------- phase D: attention over gathered keys -------------
    with tc.tile_pool(name="phD", bufs=2) as pd, \
         tc.tile_pool(name="probs", bufs=4) as pprob, \
         tc.tile_pool(name="psSc", bufs=4, space="PSUM") as psSc, \
         tc.tile_pool(name="psO", bufs=2, space="PSUM") as psO, \
         tc.tile_pool(name="psR", bufs=1, space="PSUM") as psR, \
         tc.tile_pool(name="psB2", bufs=1, space="PSUM") as psB2:
        dctx = ExitStack()
        mask_pool = None
        if with_mask:
            mask_pool = dctx.enter_context(tc.tile_pool(name="maskp", bufs=4))

        for r in range(NPANEL):
            nc.sync.dma_start(kpe2[:, r * W:(r + 1) * W],
                              kv_gath[r, :, KPE_OFF:KV_ELE])

        # deferred normalization finish of the previous head, emitted
        # late so its PE bcast / DVE reciprocal never stall the in-order
        # PE stream
        def finish_head(h, po, pr):
            rrow = pd.tile([1, W], F32, tag="rrow")
            nc.vector.reciprocal_approx_fast(rrow[:], pr[:])
            rrow_bf = pd.tile([1, W], BF16, tag="rrow_bf")
            nc.scalar.activation(rrow_bf[:], rrow[:], COPY)
            bc_ps = psB2.tile([P, W], F32, tag="bcd")
            mm(bc_ps[:], ones_row[:], rrow_bf[:], start=True, stop=True)
            bci = pd.tile([P, W], F32, tag="bci")
            nc.scalar.activation(bci[:], bc_ps[:], COPY)
            nc.vector.tensor_tensor(oT[:, h, :], po[:], bci[:], MULT)

        pending = None
        for g in range(NH // 2):
            # V for the 2 heads of this group, gathered panel-major
            v_sb = pdv.tile([P, KB_S, 2 * DV], BF16, tag="v")
            for r in range(NPANEL):
                nc.sync.dma_start(
                    v_sb[:, r * KB_W:(r + 1) * KB_W, :],
                    kv_gath[r, :, V_OFF + g * KB_W * 2 * DV:
                            V_OFF + (g + 1) * KB_W * 2 * DV])

            for hl in range(2):
                h = g * 2 + hl
                knT = pdk.tile([P, KB_S, P], BF16, tag="knT")
                for r in range(NPANEL):
                    nc.sync.dma_start(
                        knT[:, r * KB_W:(r + 1) * KB_W, :],
                        kv_gath[r, :, h * W:(h + 1) * W])

                # attention for head h, software-pipelined over kb
                po = psO.tile([P, W], F32, tag="po")
                pr = psR.tile([1, W], F32, tag="pr")
                hp64 = hl * DR
                probs_q = []
                for kb in range(KB_S):
                    pss = psSc.tile([P, W], F32, tag="pss")
                    mm(pss[:], knT[:, kb, :], qnopeT[:, h, :],
                       start=True, stop=False)
                    mm(pss[:], kpe2[hp64:hp64 + DR, kb * P:(kb + 1) * P],
                       qpeT[hp64:hp64 + DR, g, :], start=False, stop=True)
                    probs = pprob.tile([P, W], BF16, tag="probs")
                    if with_mask:
                        mtile = mask_pool.tile([P, W], F32, tag="mt")
                        nc.sync.dma_start(
                            mtile[:], t["maskT"][kb * P:(kb + 1) * P, :])
                        pf = pprob.tile([P, W], F32, tag="probs_f")
                        nc.vector.scalar_tensor_tensor(
                            pf[:], pss[:], SCALE, mtile[:], MULT, ADD)
                        nc.scalar.activation(probs[:], pf[:], EXP)
                    else:
                        nc.scalar.activation(probs[:], pss[:], EXP,
                                             scale=SCALE)
                    probs_q.append((kb, probs))
                    if len(probs_q) == 4 or kb == KB_S - 1:
                        for kb2, pb2 in probs_q:
                            mm(po[:], v_sb[:, kb2, hl * DV:(hl + 1) * DV],
                               pb2[:], start=(kb2 == 0),
                               stop=(kb2 == KB_S - 1), skip_group_check=True)
                            mm(pr[:], ones_col[:], pb2[:],
                               start=(kb2 == 0), stop=(kb2 == KB_S - 1),
                               skip_group_check=True)
                        probs_q = []
                        # finish the previous head once this head's first
                        # kb batch is in flight: its reciprocal has had
                        # time to drain, so the PE bcast never stalls
                        if pending is not None:
                            finish_head(*pending)
                            pending = None
                pending = (h, po, pr)
        finish_head(*pending)
        dctx.close()

    # ------------- phase E: o_proj -----------------------------------
    with tc.tile_pool(name="phE", bufs=2) as pe, \
         tc.tile_pool(name="psA", bufs=3, space="PSUM") as psA:
        for m in range(MB_HID):
            wm = pew.tile([P, NH, P], BF16, tag="wo")
            nc.sync.dma_start(wm[:], t["w_o"][m])
            ps = psA.tile([P, W], F32, tag="psA")
            for k in range(NH):
                mm(ps[:], wm[:, k, :], oT[:, k, :],
                   start=(k == 0), stop=(k == NH - 1))
            osb = pe.tile([P, W], F32, tag="osb")
            nc.scalar.activation(osb[:], ps[:], COPY)
            nc.sync.dma_start(t["outT"][m * P:(m + 1) * P, :], osb[:])
    top.close()


def _build_program(with_mask):
    nc = bacc.Bacc("TRN2", target_bir_lowering=False, debug=False)
    t = {"replica_groups": [list(range(b * NPANEL, (b + 1) * NPANEL))
                            for b in range(B)]}

    def inp(name, shape, dt=BF16):
        t[name] = nc.dram_tensor(name, list(shape), dt,
                                 kind="ExternalInput").ap()

    inp("hs_pkp", [P, KB_HID, W])
    inp("w_qa", [MB_QLR, P, KB_HID, P])
    inp("w_qb", [P, MB_NOPE + MB_PE, KB_QLR, P])
    inp("w_kva", [P, MB_KVA, KB_HID, P])
    inp("w_kvb_kn", [NH, P, KB_CKV, DN])
    inp("w_kvb_v", [NH // 2, P, KB_CKV, 2 * DV])
    inp("w_o", [MB_HID, P, NH, P])
    inp("qa_ln_p", [P, MB_QLR], F32)
    inp("kva_ln_p", [P, KB_CKV], F32)
    inp("cos2p", [P, W])
    inp("sin2sp", [P, W])
    if with_mask:
        inp("maskT", [S, W], F32)
    t["outT"] = nc.dram_tensor("outT", [HID, W], F32,
                               kind="ExternalOutput").ap()

    with tile.TileContext(nc) as tc:
        _emit(tc, t, with_mask)
    nc.compile()
    return nc


_PROG_CACHE = {}


def _get_program(with_mask):
    if with_mask not in _PROG_CACHE:
        _PROG_CACHE[with_mask] = _build_program(with_mask)
    return _PROG_CACHE[with_mask]


def _block4(w, mb, kb):
    """[kb*P, mb*P] -> [mb, P, kb, P] with W[m,p,k,c] = w[k*P+p, m*P+c]."""
    return np.ascontiguousarray(
        w.reshape(kb, P, mb, P).transpose(2, 1, 0, 3))


def make_in_maps(hidden_states, attention_mask, cos, sin, w_qa, qa_ln, w_qb,
                 w_kva, kva_ln, w_kvb, w_o, with_mask):
    f32 = np.float32
    c = np.ascontiguousarray

    w_qb_r = np.asarray(w_qb, f32).reshape(QLR, NH, DQK)
    w_qb_re = np.concatenate(
        [w_qb_r[:, :, :DN].reshape(QLR, NH * DN),
         w_qb_r[:, :, DN:].reshape(QLR, NH * DR)], axis=1)
    w_kva_pad = np.concatenate(
        [np.asarray(w_kva, f32), np.zeros((HID, P - DR), f32)], axis=1)
    kvb = np.asarray(w_kvb, f32).reshape(KB_CKV, P, NH, DN + DV)
    w_kvb_kn = c(kvb[:, :, :, :DN].transpose(2, 1, 0, 3)
                 .astype(NPBF))                        # [NH, P, KB_CKV, DN]
    w_kvb_v = c(kvb[:, :, :, DN:].reshape(KB_CKV, P, NH // 2, 2 * DV)
                .transpose(2, 1, 0, 3).astype(NPBF))   # [NH/2, P, kc, 256]

    qa_ln_p = c(np.asarray(qa_ln, f32).reshape(MB_QLR, P).T)
    kva_ln_p = c(np.asarray(kva_ln, f32).reshape(KB_CKV, P).T)

    cosT = np.asarray(cos, f32).T                      # [64, S]
    sinT = np.asarray(sin, f32).T
    sin_s = np.concatenate([-sinT[:DR // 2], sinT[DR // 2:]], axis=0)
    cos2 = c(np.concatenate([cosT, cosT], axis=0))     # [128, S]
    sin2s = c(np.concatenate([sin_s, sin_s], axis=0))

    shared = {
        "w_qa": _block4(np.asarray(w_qa, f32), MB_QLR, KB_HID).astype(NPBF),
        "w_qb": c(_block4(w_qb_re, MB_NOPE + MB_PE, KB_QLR)
                  .transpose(1, 0, 2, 3).astype(NPBF)),
        "w_kva": c(w_kva_pad.reshape(KB_HID, P, MB_KVA, P)
                   .transpose(1, 2, 0, 3).astype(NPBF)),
        "w_kvb_kn": w_kvb_kn,
        "w_kvb_v": w_kvb_v,
        "w_o": _block4(np.asarray(w_o, f32), MB_HID, KB_HID).astype(NPBF),
        "qa_ln_p": qa_ln_p,
        "kva_ln_p": kva_ln_p,
    }

    hs = np.asarray(hidden_states)
    am = np.asarray(attention_mask)
    in_maps = []
    for core in range(NCORES):
        b, pnl = divmod(core, NPANEL)
        q0 = pnl * W
        hsT = np.asarray(hs[b], f32).T[:, q0:q0 + W]   # [HID, W]
        m = dict(shared)
        m["hs_pkp"] = c(hsT.reshape(KB_HID, P, W).transpose(1, 0, 2)
                        .astype(NPBF))                 # [128, 16, W]
        m["cos2p"] = c(cos2[:, q0:q0 + W].astype(NPBF))
        m["sin2sp"] = c(sin2s[:, q0:q0 + W].astype(NPBF))
        if with_mask:
            m["maskT"] = c(am[b, 0, q0:q0 + W, :].T.astype(f32))
        in_maps.append(m)
    return in_maps


def kernel(hidden_states, attention_mask, cos, sin, w_qa, qa_ln, w_qb,
           w_kva, kva_ln, w_kvb, w_o):
    global LAST_RESULT
    with_mask = bool(np.any(np.asarray(attention_mask) != 0))
    nc = _get_program(with_mask)
    in_maps = make_in_maps(hidden_states, attention_mask, cos, sin, w_qa,
                           qa_ln, w_qb, w_kva, kva_ln, w_kvb, w_o, with_mask)
    trace = os.environ.get("KERNEL_TRACE", "0") == "1"
    res = bass_utils.run_bass_kernel_spmd(
        nc, in_maps, core_ids=list(range(NCORES)), trace=trace)
    LAST_RESULT = res

    out = np.empty((B, S, HID), np.float32)
    for core in range(NCORES):
        b, pnl = divmod(core, NPANEL)
        q0 = pnl * W
        out[b, q0:q0 + W, :] = res.results[core]["outT"].T
    return out


# revision 63
# speedup vs baseline: 1.0828x; 1.0075x over previous
"""DeepseekV2 MLA attention forward — Trainium2 Bass kernel (8 NeuronCores).

Sharding: data-parallel over batch (2) x sequence-parallel over query rows
(4 panels of 512) = 8 cores.  Each core:
  - computes the kv path (kv_a -> rmsnorm -> kv_b, k_pe rope) for its OWN
    512-row panel only (identical SPMD instructions on every core),
  - AllGathers (k_nope, V, roped k_pe) across the 4 cores of its batch,
    giving every core the full key set in panel-major (= original) order,
  - computes the q path for its panel, full attention over the gathered
    keys for all 16 heads, and o_proj.
Attention is key-permutation invariant, so gathering in rank order needs
no reordering.  The gather overlaps the q_a/q_b projections.

All matmul operands are bf16 (fp32 PSUM accumulation).  Weights are
pre-transposed on the host into per-partition-contiguous [m][p][k][c]
blocks so every weight DMA is one large contiguous burst.  Intermediates
stay resident in SBUF.  The attention kb loop is software-pipelined so
the PE never waits on the Act-engine exp; softmax normalization uses
reciprocal_approx_fast and is deferred one head so the PE never stalls.
"""

import os
import numpy as np
import ml_dtypes
from contextlib import ExitStack

import concourse.bass as bass
import concourse.bacc as bacc
import concourse.mybir as mybir
import concourse.tile as tile
from concourse import bass_utils

B, S, HID = 2, 2048, 2048
NH = 16
QLR, KVLR = 1536, 512
DN, DR, DV = 128, 64, 128
DQK = DN + DR
SCALE = DQK ** -0.5
EPS = 1e-6
P = 128
NPANEL = 4
W = S // NPANEL            # 512 query rows per core
NCORES = B * NPANEL

F32 = mybir.dt.float32
F32R = mybir.dt.float32r
BF16 = mybir.dt.bfloat16
NPBF = ml_dtypes.bfloat16
EXP = mybir.ActivationFunctionType.Exp
SQRT = mybir.ActivationFunctionType.Sqrt
SQUARE = mybir.ActivationFunctionType.Square
COPY = mybir.ActivationFunctionType.Copy
MULT = mybir.AluOpType.mult
ADD = mybir.AluOpType.add

KB_HID = HID // P          # 16
KB_QLR = QLR // P          # 12
KB_CKV = KVLR // P         # 4
KB_S = S // P              # 16
KB_W = W // P              # 4  key blocks in own panel
MB_QLR = QLR // P          # 12
MB_KVA = 5                 # 4 ckv blocks + 1 (zero-padded) rope block
MB_NOPE = NH * DN // P     # 16
MB_PE = NH * DR // P       # 8
MB_HID = HID // P          # 16

# gathered kv payload layout (bf16 elements per partition)
KN_ELE = NH * W            # 8192  k_nope  [h][key]
V_ELE = (NH // 2) * KB_W * 2 * DV   # 8192  V  [g][kb_own][2*dv]
KPE_ELE = W                # 512   roped k_pe (2x64 duplicated rows)
KV_ELE = KN_ELE + V_ELE + KPE_ELE   # 16896
V_OFF = KN_ELE
KPE_OFF = KN_ELE + V_ELE

LAST_RESULT = None         # BassKernelResults of the most recent launch


def _emit(tc, t, with_mask):
    nc = tc.nc
    mm = nc.tensor.matmul
    top = ExitStack()

    const = top.enter_context(tc.tile_pool(name="const", bufs=1))
    ones_col = const.tile([P, 1], BF16)
    nc.vector.memset(ones_col[:], 1.0)
    ones_row = const.tile([1, P], BF16)
    nc.vector.memset(ones_row[:], 1.0)
    eps1 = const.tile([1, 1], F32)
    nc.vector.memset(eps1[:], EPS)
    qa_ln = const.tile([P, MB_QLR], F32)
    nc.sync.dma_start(qa_ln[:], t["qa_ln_p"][:])
    kva_ln = const.tile([P, KB_CKV], F32)
    nc.sync.dma_start(kva_ln[:], t["kva_ln_p"][:])
    cos2p = const.tile([P, W], BF16)
    nc.sync.dma_start(cos2p[:], t["cos2p"][:])
    sin2sp = const.tile([P, W], BF16)
    nc.sync.dma_start(sin2sp[:], t["sin2sp"][:])

    # persistent SBUF intermediates (all bf16)
    persist = top.enter_context(tc.tile_pool(name="persist", bufs=1))
    qaT = persist.tile([P, MB_QLR, W], BF16)       # q_a output, normalized
    kpe2 = persist.tile([P, S], BF16)              # gathered roped k_pe
    qnopeT = persist.tile([P, MB_NOPE, W], BF16)
    qpeT = persist.tile([P, MB_PE, W], BF16)       # roped q_pe
    oT = persist.tile([P, NH, W], BF16)            # attn out (pre-o_proj)

    # Weight pools for later phases sit BELOW the per-phase scratch pools
    # in the SBUF stack, so their prefetch DMAs never carry a write-after-
    # read hazard against the previous phase's scratch tiles.  w_qb is
    # preloaded whole so phase C has no DMA dependence at all once the
    # collective starts competing for DMA queues.
    pqb = top.enter_context(tc.tile_pool(name="wqb_all", bufs=1))
    wqb = pqb.tile([P, MB_NOPE + MB_PE, KB_QLR, P], BF16)

    # own-panel hidden states, prefetched first
    ph = top.enter_context(tc.tile_pool(name="hpanel", bufs=1))
    hn0 = ph.tile([P, KB_HID, W], BF16)
    nc.sync.dma_start(hn0[:], t["hs_pkp"][:])
    paw = top.enter_context(tc.tile_pool(name="phA_w", bufs=3))
    pkw = top.enter_context(tc.tile_pool(name="kvb_w", bufs=2))
    dram = top.enter_context(tc.tile_pool(name="dram", bufs=1, space="DRAM"))
    kv_out = dram.tile([P, KV_ELE], BF16)
    kv_gath = dram.tile([NPANEL, P, KV_ELE], BF16)

    def rsqrt_bcast(pool, psum_pool, ss_ps, inv_dim):
        """[1,n] sum-of-squares psum -> [P,n] f32 PSUM of 1/sqrt(mean+eps)."""
        n = ss_ps.shape[-1]
        srow = pool.tile([1, n], F32, tag="srow")
        nc.scalar.activation(srow[:], ss_ps[:], SQRT, bias=eps1[:],
                             scale=inv_dim)
        rrow = pool.tile([1, n], F32, tag="rrow")
        nc.vector.reciprocal_approx_fast(rrow[:], srow[:])
        rrow_bf = pool.tile([1, n], BF16, tag="rrow_bf")
        nc.scalar.activation(rrow_bf[:], rrow[:], COPY)
        bc_ps = psum_pool.tile([P, n], F32, tag="bcast")
        mm(bc_ps[:], ones_row[:], rrow_bf[:], start=True, stop=True)
        return bc_ps

    # ------------- phase B: own-panel kv_a + rmsnorm + kpe rope ------
    with tc.tile_pool(name="phB", bufs=2) as pb, \
         tc.tile_pool(name="phB_w", bufs=2) as pbw, \
         tc.tile_pool(name="kvbuf", bufs=1) as pkv, \
         tc.tile_pool(name="psA", bufs=3, space="PSUM") as psA, \
         tc.tile_pool(name="psS", bufs=1, space="PSUM") as psSS, \
         tc.tile_pool(name="psB", bufs=1, space="PSUM") as psBC:
        ckT = pkv.tile([P, KB_CKV, W], BF16)       # own compressed kv
        kpo = pkv.tile([P, W], BF16)               # own roped k_pe

        ss = psSS.tile([1, W], F32, tag="ss")
        kp = pb.tile([P, W], BF16, tag="kp")
        sq_prev = None
        for m in range(MB_KVA):
            wkva = pbw.tile([P, KB_HID, P], BF16, tag="wkva")
            nc.sync.dma_start(wkva[:], t["w_kva"][:, m])
            ps = psA.tile([P, W], F32, tag="psA")
            for k in range(KB_HID):
                mm(ps[:], wkva[:, k, :], hn0[:, k, :],
                   start=(k == 0), stop=(k == KB_HID - 1))
            if m < KB_CKV:
                nc.scalar.activation(ckT[:, m, :], ps[:], COPY)
                sq = pb.tile([P, W], BF16, tag="sq")
                nc.scalar.activation(sq[:], ps[:], SQUARE)
                if sq_prev is not None:
                    mm(ss[:], ones_col[:], sq_prev,
                       start=(m == 1), stop=False, skip_group_check=True)
                sq_prev = sq[:]
            else:
                mm(ss[:], ones_col[:], sq_prev, start=False, stop=True,
                   skip_group_check=True)
                nc.scalar.activation(kp[0:DR, :], ps[0:DR, :], COPY)
                nc.vector.tensor_copy(kp[DR:P, :], ps[0:DR, :])
        rk = rsqrt_bcast(pb, psBC, ss[:], 1.0 / KVLR)
        for m in range(KB_CKV):
            nc.vector.scalar_tensor_tensor(
                ckT[:, m, :], ckT[:, m, :], kva_ln[:, m:m + 1], rk[:],
                MULT, MULT)
        # RoPE on kp (both 64-halves hold the same data)
        rot = pb.tile([P, W], BF16, tag="rot")
        for h in (0, DR):
            nc.vector.tensor_copy(rot[h:h + 32, :], kp[h + 32:h + 64, :])
            nc.vector.tensor_copy(rot[h + 32:h + 64, :], kp[h:h + 32, :])
        nc.vector.tensor_tensor(kp[:], kp[:], cos2p[:], MULT)
        nc.vector.tensor_tensor(rot[:], rot[:], sin2sp[:], MULT)
        nc.vector.tensor_tensor(kpo[:], kp[:], rot[:], ADD)

        # ------------- phase B2: own-panel kv_b (k_nope + V) ---------
        # outputs stream straight to the DRAM collective bounce buffer
        with tc.tile_pool(name="psK", bufs=3, space="PSUM") as psK, \
             tc.tile_pool(name="kvstage", bufs=3) as pst:
            for h in range(NH):
                wkn = pkw.tile([P, KB_CKV, DN], BF16, tag="wkn")
                nc.sync.dma_start(wkn[:], t["w_kvb_kn"][h])
                psk = psK.tile([P, W], F32, tag="psk")
                for kc in range(KB_CKV):
                    mm(psk[:], wkn[:, kc, :], ckT[:, kc, :],
                       start=(kc == 0), stop=(kc == KB_CKV - 1))
                kst = pst.tile([P, W], BF16, tag="kst")
                nc.scalar.activation(kst[:], psk[:], COPY)
                nc.sync.dma_start(kv_out[:, h * W:(h + 1) * W], kst[:])
            for g in range(NH // 2):
                wv = pkw.tile([P, KB_CKV, 2 * DV], BF16, tag="wv")
                nc.sync.dma_start(wv[:], t["w_kvb_v"][g])
                for kb in range(KB_W):
                    psv = psK.tile([P, W], F32, tag="psk")
                    for kc in range(KB_CKV):
                        mm(psv[:, :2 * DV],
                           ckT[:, kc, kb * P:(kb + 1) * P], wv[:, kc, :],
                           start=(kc == 0), stop=(kc == KB_CKV - 1))
                    vst = pst.tile([P, 2 * DV], BF16, tag="vst")
                    nc.scalar.activation(vst[:], psv[:, :2 * DV], COPY)
                    nc.sync.dma_start(
                        kv_out[:, V_OFF + (g * KB_W + kb) * 2 * DV:
                               V_OFF + (g * KB_W + kb + 1) * 2 * DV],
                        vst[:])
            nc.sync.dma_start(kv_out[:, KPE_OFF:KV_ELE], kpo[:])

    # gather the batch group's full kv set; emitted straight after the
    # kv_out writes so its (dominance-based) semaphore wait covers only
    # them — it fires while the q path computes
    nc.gpsimd.collective_compute(
        "AllGather", mybir.AluOpType.bypass,
        replica_groups=t["replica_groups"],
        ins=[kv_out.opt()], outs=[kv_gath.opt()])

    # preload ALL of w_qb; phase C then has no DMA dependence at all
    nc.sync.dma_start(wqb[:], t["w_qb"][:])

    # phase D DMA-destination pools (opened after B's scratch is freed,
    # before C's scratch, so D's gathered-kv DMAs carry no WAR hazard)
    pdv = top.enter_context(tc.tile_pool(name="phD_v", bufs=1))
    pdk = top.enter_context(tc.tile_pool(name="phD_k", bufs=2))
    pew = top.enter_context(tc.tile_pool(name="phE_w", bufs=2))

    # ------------- phase A: qaT panel + rmsnorm ----------------------
    with tc.tile_pool(name="phA", bufs=2) as pa, \
         tc.tile_pool(name="psA", bufs=3, space="PSUM") as psA, \
         tc.tile_pool(name="psS", bufs=1, space="PSUM") as psSS, \
         tc.tile_pool(name="psB", bufs=1, space="PSUM") as psBC:
        ss = psSS.tile([1, W], F32, tag="ss")
        sq_prev = None
        for m in range(MB_QLR):
            wm = paw.tile([P, KB_HID, P], BF16, tag="wqa")
            nc.sync.dma_start(wm[:], t["w_qa"][m])
            ps = psA.tile([P, W], F32, tag="psA")
            for k in range(KB_HID):
                mm(ps[:], wm[:, k, :], hn0[:, k, :],
                   start=(k == 0), stop=(k == KB_HID - 1))
            nc.scalar.activation(qaT[:, m, :], ps[:], COPY)
            sq = pa.tile([P, W], BF16, tag="sq")
            nc.scalar.activation(sq[:], ps[:], SQUARE)
            if sq_prev is not None:
                mm(ss[:], ones_col[:], sq_prev,
                   start=(m == 1), stop=False, skip_group_check=True)
            sq_prev = sq[:]
        mm(ss[:], ones_col[:], sq_prev, start=False, stop=True,
           skip_group_check=True)
        rq = rsqrt_bcast(pa, psBC, ss[:], 1.0 / QLR)
        for m in range(MB_QLR):
            nc.vector.scalar_tensor_tensor(
                qaT[:, m, :], qaT[:, m, :], qa_ln[:, m:m + 1], rq[:],
                MULT, MULT)

    # ------------- phase C: q_b panel (+ RoPE on pe part) ------------
    with tc.tile_pool(name="phC", bufs=2) as pc, \
         tc.tile_pool(name="psA", bufs=3, space="PSUM") as psA:
        for m in range(MB_NOPE + MB_PE):
            ps = psA.tile([P, W], F32, tag="psA")
            for k in range(KB_QLR):
                mm(ps[:], wqb[:, m, k, :], qaT[:, k, :],
                   start=(k == 0), stop=(k == KB_QLR - 1))
            if m < MB_NOPE:
                nc.scalar.activation(qnopeT[:, m, :], ps[:], COPY)
            else:
                j = m - MB_NOPE
                qb = pc.tile([P, W], BF16, tag="qb")
                nc.scalar.activation(qb[:], ps[:], COPY)
                rotq = pc.tile([P, W], BF16, tag="rotq")
                for h in (0, DR):
                    nc.vector.tensor_copy(rotq[h:h + 32, :],
                                          qb[h + 32:h + 64, :])
                    nc.vector.tensor_copy(rotq[h + 32:h + 64, :],
                                          qb[h:h + 32, :])
                nc.vector.tensor_tensor(rotq[:], rotq[:], sin2sp[:], MULT)
                nc.vector.tensor_tensor(qpeT[:, j, :], qb[:], cos2p[:], MULT)
                nc.vector.tensor_tensor(qpeT[:, j, :], qpeT[:, j, :],
                                        rotq[:], ADD)

    # ------------- phase D: attention over gathered keys -------------
    with tc.tile_pool(name="phD", bufs=2) as pd, \
         tc.tile_pool(name="probs", bufs=4) as pprob, \
         tc.tile_pool(name="psSc", bufs=4, space="PSUM") as psSc, \
         tc.tile_pool(name="psO", bufs=2, space="PSUM") as psO, \
         tc.tile_pool(name="psR", bufs=1, space="PSUM") as psR, \
         tc.tile_pool(name="psB2", bufs=1, space="PSUM") as psB2:
        dctx = ExitStack()
        mask_pool = None
        if with_mask:
            mask_pool = dctx.enter_context(tc.tile_pool(name="maskp", bufs=4))

        for r in range(NPANEL):
            nc.sync.dma_start(kpe2[:, r * W:(r + 1) * W],
                              kv_gath[r, :, KPE_OFF:KV_ELE])

        # deferred normalization finish of the previous head, emitted
        # late so its PE bcast / DVE reciprocal never stall the in-order
        # PE stream
        def finish_head(h, po, pr):
            rrow = pd.tile([1, W], F32, tag="rrow")
            nc.vector.reciprocal_approx_fast(rrow[:], pr[:])
            rrow_bf = pd.tile([1, W], BF16, tag="rrow_bf")
            nc.scalar.activation(rrow_bf[:], rrow[:], COPY)
            bc_ps = psB2.tile([P, W], F32, tag="bcd")
            mm(bc_ps[:], ones_row[:], rrow_bf[:], start=True, stop=True)
            bci = pd.tile([P, W], F32, tag="bci")
            nc.scalar.activation(bci[:], bc_ps[:], COPY)
            nc.vector.tensor_tensor(oT[:, h, :], po[:], bci[:], MULT)

        pending = None
        for g in range(NH // 2):
            # V for the 2 heads of this group, gathered panel-major
            v_sb = pdv.tile([P, KB_S, 2 * DV], BF16, tag="v")
            for r in range(NPANEL):
                nc.sync.dma_start(
                    v_sb[:, r * KB_W:(r + 1) * KB_W, :],
                    kv_gath[r, :, V_OFF + g * KB_W * 2 * DV:
                            V_OFF + (g + 1) * KB_W * 2 * DV])

            for hl in range(2):
                h = g * 2 + hl
                knT = pdk.tile([P, KB_S, P], BF16, tag="knT")
                for r in range(NPANEL):
                    nc.sync.dma_start(
                        knT[:, r * KB_W:(r + 1) * KB_W, :],
                        kv_gath[r, :, h * W:(h + 1) * W])

                # attention for head h, software-pipelined over kb
                po = psO.tile([P, W], F32, tag="po")
                pr = psR.tile([1, W], F32, tag="pr")
                hp64 = hl * DR
                probs_q = []
                for kb in range(KB_S):
                    pss = psSc.tile([P, W], F32, tag="pss")
                    mm(pss[:], knT[:, kb, :], qnopeT[:, h, :],
                       start=True, stop=False)
                    mm(pss[:], kpe2[hp64:hp64 + DR, kb * P:(kb + 1) * P],
                       qpeT[hp64:hp64 + DR, g, :], start=False, stop=True)
                    probs = pprob.tile([P, W], BF16, tag="probs")
                    if with_mask:
                        mtile = mask_pool.tile([P, W], F32, tag="mt")
                        nc.sync.dma_start(
                            mtile[:], t["maskT"][kb * P:(kb + 1) * P, :])
                        pf = pprob.tile([P, W], F32, tag="probs_f")
                        nc.vector.scalar_tensor_tensor(
                            pf[:], pss[:], SCALE, mtile[:], MULT, ADD)
                        nc.scalar.activation(probs[:], pf[:], EXP)
                    else:
                        nc.scalar.activation(probs[:], pss[:], EXP,
                                             scale=SCALE)
                    probs_q.append((kb, probs))
                    if len(probs_q) == 4 or kb == KB_S - 1:
                        for kb2, pb2 in probs_q:
                            mm(po[:], v_sb[:, kb2, hl * DV:(hl + 1) * DV],
                               pb2[:], start=(kb2 == 0),
                               stop=(kb2 == KB_S - 1), skip_group_check=True)
                            mm(pr[:], ones_col[:], pb2[:],
                               start=(kb2 == 0), stop=(kb2 == KB_S - 1),
                               skip_group_check=True)
                        probs_q = []
                        # finish the previous head once this head's first
                        # kb batch is in flight: its reciprocal has had
                        # time to drain, so the PE bcast never stalls
                        if pending is not None:
                            finish_head(*pending)
                            pending = None
                pending = (h, po, pr)
        finish_head(*pending)
        dctx.close()

    # ------------- phase E: o_proj -----------------------------------
    with tc.tile_pool(name="phE", bufs=2) as pe, \
         tc.tile_pool(name="psA", bufs=3, space="PSUM") as psA:
        for m in range(MB_HID):
            wm = pew.tile([P, NH, P], BF16, tag="wo")
            nc.sync.dma_start(wm[:], t["w_o"][m])
            ps = psA.tile([P, W], F32, tag="psA")
            for k in range(NH):
                mm(ps[:], wm[:, k, :], oT[:, k, :],
                   start=(k == 0), stop=(k == NH - 1))
            osb = pe.tile([P, W], F32, tag="osb")
            nc.scalar.activation(osb[:], ps[:], COPY)
            nc.sync.dma_start(t["outT"][m * P:(m + 1) * P, :], osb[:])
    top.close()


def _build_program(with_mask):
    nc = bacc.Bacc("TRN2", target_bir_lowering=False, debug=False)
    t = {"replica_groups": [list(range(b * NPANEL, (b + 1) * NPANEL))
                            for b in range(B)]}

    def inp(name, shape, dt=BF16):
        t[name] = nc.dram_tensor(name, list(shape), dt,
                                 kind="ExternalInput").ap()

    inp("hs_pkp", [P, KB_HID, W])
    inp("w_qa", [MB_QLR, P, KB_HID, P])
    inp("w_qb", [P, MB_NOPE + MB_PE, KB_QLR, P])
    inp("w_kva", [P, MB_KVA, KB_HID, P])
    inp("w_kvb_kn", [NH, P, KB_CKV, DN])
    inp("w_kvb_v", [NH // 2, P, KB_CKV, 2 * DV])
    inp("w_o", [MB_HID, P, NH, P])
    inp("qa_ln_p", [P, MB_QLR], F32)
    inp("kva_ln_p", [P, KB_CKV], F32)
    inp("cos2p", [P, W])
    inp("sin2sp", [P, W])
    if with_mask:
        inp("maskT", [S, W], F32)
    t["outT"] = nc.dram_tensor("outT", [HID, W], F32,
                               kind="ExternalOutput").ap()

    with tile.TileContext(nc) as tc:
        _emit(tc, t, with_mask)
    nc.compile()
    return nc


_PROG_CACHE = {}


def _get_program(with_mask):
    if with_mask not in _PROG_CACHE:
        _PROG_CACHE[with_mask] = _build_program(with_mask)
    return _PROG_CACHE[with_mask]


def _block4(w, mb, kb):
    """[kb*P, mb*P] -> [mb, P, kb, P] with W[m,p,k,c] = w[k*P+p, m*P+c]."""
    return np.ascontiguousarray(
        w.reshape(kb, P, mb, P).transpose(2, 1, 0, 3))


def make_in_maps(hidden_states, attention_mask, cos, sin, w_qa, qa_ln, w_qb,
                 w_kva, kva_ln, w_kvb, w_o, with_mask):
    f32 = np.float32
    c = np.ascontiguousarray

    w_qb_r = np.asarray(w_qb, f32).reshape(QLR, NH, DQK)
    w_qb_re = np.concatenate(
        [w_qb_r[:, :, :DN].reshape(QLR, NH * DN),
         w_qb_r[:, :, DN:].reshape(QLR, NH * DR)], axis=1)
    w_kva_pad = np.concatenate(
        [np.asarray(w_kva, f32), np.zeros((HID, P - DR), f32)], axis=1)
    kvb = np.asarray(w_kvb, f32).reshape(KB_CKV, P, NH, DN + DV)
    w_kvb_kn = c(kvb[:, :, :, :DN].transpose(2, 1, 0, 3)
                 .astype(NPBF))                        # [NH, P, KB_CKV, DN]
    w_kvb_v = c(kvb[:, :, :, DN:].reshape(KB_CKV, P, NH // 2, 2 * DV)
                .transpose(2, 1, 0, 3).astype(NPBF))   # [NH/2, P, kc, 256]

    qa_ln_p = c(np.asarray(qa_ln, f32).reshape(MB_QLR, P).T)
    kva_ln_p = c(np.asarray(kva_ln, f32).reshape(KB_CKV, P).T)

    cosT = np.asarray(cos, f32).T                      # [64, S]
    sinT = np.asarray(sin, f32).T
    sin_s = np.concatenate([-sinT[:DR // 2], sinT[DR // 2:]], axis=0)
    cos2 = c(np.concatenate([cosT, cosT], axis=0))     # [128, S]
    sin2s = c(np.concatenate([sin_s, sin_s], axis=0))

    shared = {
        "w_qa": _block4(np.asarray(w_qa, f32), MB_QLR, KB_HID).astype(NPBF),
        "w_qb": c(_block4(w_qb_re, MB_NOPE + MB_PE, KB_QLR)
                  .transpose(1, 0, 2, 3).astype(NPBF)),
        "w_kva": c(w_kva_pad.reshape(KB_HID, P, MB_KVA, P)
                   .transpose(1, 2, 0, 3).astype(NPBF)),
        "w_kvb_kn": w_kvb_kn,
        "w_kvb_v": w_kvb_v,
        "w_o": _block4(np.asarray(w_o, f32), MB_HID, KB_HID).astype(NPBF),
        "qa_ln_p": qa_ln_p,
        "kva_ln_p": kva_ln_p,
    }

    hs = np.asarray(hidden_states)
    am = np.asarray(attention_mask)
    in_maps = []
    for core in range(NCORES):
        b, pnl = divmod(core, NPANEL)
        q0 = pnl * W
        hsT = np.asarray(hs[b], f32).T[:, q0:q0 + W]   # [HID, W]
        m = dict(shared)
        m["hs_pkp"] = c(hsT.reshape(KB_HID, P, W).transpose(1, 0, 2)
                        .astype(NPBF))                 # [128, 16, W]
        m["cos2p"] = c(cos2[:, q0:q0 + W].astype(NPBF))
        m["sin2sp"] = c(sin2s[:, q0:q0 + W].astype(NPBF))
        if with_mask:
            m["maskT"] = c(am[b, 0, q0:q0 + W, :].T.astype(f32))
        in_maps.append(m)
    return in_maps


def kernel(hidden_states, attention_mask, cos, sin, w_qa, qa_ln, w_qb,
           w_kva, kva_ln, w_kvb, w_o):
    global LAST_RESULT
    with_mask = bool(np.any(np.asarray(attention_mask) != 0))
    nc = _get_program(with_mask)
    in_maps = make_in_maps(hidden_states, attention_mask, cos, sin, w_qa,
                           qa_ln, w_qb, w_kva, kva_ln, w_kvb, w_o, with_mask)
    trace = os.environ.get("KERNEL_TRACE", "0") == "1"
    res = bass_utils.run_bass_kernel_spmd(
        nc, in_maps, core_ids=list(range(NCORES)), trace=trace)
    LAST_RESULT = res

    out = np.empty((B, S, HID), np.float32)
    for core in range(NCORES):
        b, pnl = divmod(core, NPANEL)
        q0 = pnl * W
        out[b, q0:q0 + W, :] = res.results[core]["outT"].T
    return out


# revision 73
# speedup vs baseline: 1.1276x; 1.0414x over previous
"""DeepseekV2 MLA attention forward — Trainium2 Bass kernel (8 NeuronCores).

Sharding: data-parallel over batch (2) x sequence-parallel over query rows
(4 panels of 512) = 8 cores.  Each core:
  - computes the kv path (kv_a -> rmsnorm -> kv_b, k_pe rope) for its OWN
    512-row panel only (identical SPMD instructions on every core),
  - AllGathers (k_nope, V, roped k_pe) across the 4 cores of its batch,
    giving every core the full key set in panel-major (= original) order,
  - computes the q path for its panel, full attention over the gathered
    keys for all 16 heads, and o_proj.
Attention is key-permutation invariant, so gathering in rank order needs
no reordering.  The gather overlaps the q_a/q_b projections.

All matmul operands are bf16 (fp32 PSUM accumulation).  Weights are
pre-transposed on the host into per-partition-contiguous [m][p][k][c]
blocks so every weight DMA is one large contiguous burst.  Intermediates
stay resident in SBUF.  The attention kb loop is software-pipelined so
the PE never waits on the Act-engine exp; softmax normalization uses
reciprocal_approx_fast and is deferred one head so the PE never stalls.
"""

import os
import numpy as np
import ml_dtypes
from contextlib import ExitStack

import concourse.bass as bass
import concourse.bacc as bacc
import concourse.mybir as mybir
import concourse.tile as tile
from concourse import bass_utils

B, S, HID = 2, 2048, 2048
NH = 16
QLR, KVLR = 1536, 512
DN, DR, DV = 128, 64, 128
DQK = DN + DR
SCALE = DQK ** -0.5
EPS = 1e-6
P = 128
NPANEL = 4
W = S // NPANEL            # 512 query rows per core
NCORES = B * NPANEL

F32 = mybir.dt.float32
F32R = mybir.dt.float32r
BF16 = mybir.dt.bfloat16
NPBF = ml_dtypes.bfloat16
EXP = mybir.ActivationFunctionType.Exp
SQRT = mybir.ActivationFunctionType.Sqrt
SQUARE = mybir.ActivationFunctionType.Square
COPY = mybir.ActivationFunctionType.Copy
MULT = mybir.AluOpType.mult
ADD = mybir.AluOpType.add

KB_HID = HID // P          # 16
KB_QLR = QLR // P          # 12
KB_CKV = KVLR // P         # 4
KB_S = S // P              # 16
KB_W = W // P              # 4  key blocks in own panel
MB_QLR = QLR // P          # 12
MB_KVA = 5                 # 4 ckv blocks + 1 (zero-padded) rope block
MB_NOPE = NH * DN // P     # 16
MB_PE = NH * DR // P       # 8
MB_HID = HID // P          # 16

# gathered kv payload layout (bf16 elements per partition): the
# compressed latents only — kv_b is recomputed locally for the full S
# (the AllGather runs at ~40 GB/s, so shipping expanded k_nope/V would
# cost far more than recomputing them)
KPE_OFF = KB_CKV * W       # 2048  normalized ckv  [kc][key]
KV_ELE = KPE_OFF + W       # 2560  + roped k_pe (2x64 duplicated rows)

LAST_RESULT = None         # BassKernelResults of the most recent launch


def _emit(tc, t, with_mask):
    nc = tc.nc
    mm = nc.tensor.matmul
    top = ExitStack()

    const = top.enter_context(tc.tile_pool(name="const", bufs=1))
    ones_col = const.tile([P, 1], BF16)
    nc.vector.memset(ones_col[:], 1.0)
    ones_row = const.tile([1, P], BF16)
    nc.vector.memset(ones_row[:], 1.0)
    eps1 = const.tile([1, 1], F32)
    nc.vector.memset(eps1[:], EPS)
    qa_ln = const.tile([P, MB_QLR], F32)
    nc.sync.dma_start(qa_ln[:], t["qa_ln_p"][:])
    kva_ln = const.tile([P, KB_CKV], F32)
    nc.sync.dma_start(kva_ln[:], t["kva_ln_p"][:])
    cos2p = const.tile([P, W], BF16)
    nc.sync.dma_start(cos2p[:], t["cos2p"][:])
    sin2sp = const.tile([P, W], BF16)
    nc.sync.dma_start(sin2sp[:], t["sin2sp"][:])

    # persistent SBUF intermediates (all bf16)
    persist = top.enter_context(tc.tile_pool(name="persist", bufs=1))
    qaT = persist.tile([P, MB_QLR, W], BF16)       # q_a output, normalized
    kpe2 = persist.tile([P, S], BF16)              # gathered roped k_pe
    ckTf = persist.tile([P, KB_CKV, S], BF16)      # gathered norm. ckv
    qnopeT = persist.tile([P, MB_NOPE, W], BF16)
    qpeT = persist.tile([P, MB_PE, W], BF16)       # roped q_pe
    oT = persist.tile([P, NH, W], BF16)            # attn out (pre-o_proj)

    # Weight pools for later phases sit BELOW the per-phase scratch pools
    # in the SBUF stack, so their prefetch DMAs never carry a write-after-
    # read hazard against the previous phase's scratch tiles.  w_qb is
    # preloaded whole so phase C has no DMA dependence at all once the
    # collective starts competing for DMA queues.
    pcw = top.enter_context(tc.tile_pool(name="phC_w", bufs=2))

    # own-panel hidden states, prefetched first
    ph = top.enter_context(tc.tile_pool(name="hpanel", bufs=1))
    hn0 = ph.tile([P, KB_HID, W], BF16)
    nc.sync.dma_start(hn0[:], t["hs_pkp"][:])
    paw = top.enter_context(tc.tile_pool(name="phA_w", bufs=3))
    pkw = top.enter_context(tc.tile_pool(name="kvb_w", bufs=2))
    dram = top.enter_context(tc.tile_pool(name="dram", bufs=1, space="DRAM"))
    kv_out = dram.tile([P, KV_ELE], BF16)
    kv_gath = dram.tile([NPANEL, P, KV_ELE], BF16)

    def rsqrt_bcast(pool, psum_pool, ss_ps, inv_dim):
        """[1,n] sum-of-squares psum -> [P,n] f32 PSUM of 1/sqrt(mean+eps)."""
        n = ss_ps.shape[-1]
        srow = pool.tile([1, n], F32, tag="srow")
        nc.scalar.activation(srow[:], ss_ps[:], SQRT, bias=eps1[:],
                             scale=inv_dim)
        rrow = pool.tile([1, n], F32, tag="rrow")
        nc.vector.reciprocal_approx_fast(rrow[:], srow[:])
        rrow_bf = pool.tile([1, n], BF16, tag="rrow_bf")
        nc.scalar.activation(rrow_bf[:], rrow[:], COPY)
        bc_ps = psum_pool.tile([P, n], F32, tag="bcast")
        mm(bc_ps[:], ones_row[:], rrow_bf[:], start=True, stop=True)
        return bc_ps

    # ------------- phase B: own-panel kv_a + rmsnorm + kpe rope ------
    with tc.tile_pool(name="phB", bufs=2) as pb, \
         tc.tile_pool(name="phB_w", bufs=2) as pbw, \
         tc.tile_pool(name="kvbuf", bufs=1) as pkv, \
         tc.tile_pool(name="psA", bufs=3, space="PSUM") as psA, \
         tc.tile_pool(name="psS", bufs=1, space="PSUM") as psSS, \
         tc.tile_pool(name="psB", bufs=1, space="PSUM") as psBC:
        ckT = pkv.tile([P, KB_CKV, W], BF16)       # own compressed kv
        kpo = pkv.tile([P, W], BF16)               # own roped k_pe

        ss = psSS.tile([1, W], F32, tag="ss")
        kp = pb.tile([P, W], BF16, tag="kp")
        sq_prev = None
        for m in range(MB_KVA):
            wkva = pbw.tile([P, KB_HID, P], BF16, tag="wkva")
            nc.sync.dma_start(wkva[:], t["w_kva"][:, m])
            ps = psA.tile([P, W], F32, tag="psA")
            for k in range(KB_HID):
                mm(ps[:], wkva[:, k, :], hn0[:, k, :],
                   start=(k == 0), stop=(k == KB_HID - 1))
            if m < KB_CKV:
                nc.scalar.activation(ckT[:, m, :], ps[:], COPY)
                sq = pb.tile([P, W], BF16, tag="sq")
                nc.scalar.activation(sq[:], ps[:], SQUARE)
                if sq_prev is not None:
                    mm(ss[:], ones_col[:], sq_prev,
                       start=(m == 1), stop=False, skip_group_check=True)
                sq_prev = sq[:]
            else:
                mm(ss[:], ones_col[:], sq_prev, start=False, stop=True,
                   skip_group_check=True)
                nc.scalar.activation(kp[0:DR, :], ps[0:DR, :], COPY)
                nc.vector.tensor_copy(kp[DR:P, :], ps[0:DR, :])
        rk = rsqrt_bcast(pb, psBC, ss[:], 1.0 / KVLR)
        for m in range(KB_CKV):
            nc.vector.scalar_tensor_tensor(
                ckT[:, m, :], ckT[:, m, :], kva_ln[:, m:m + 1], rk[:],
                MULT, MULT)
        # RoPE on kp (both 64-halves hold the same data)
        rot = pb.tile([P, W], BF16, tag="rot")
        for h in (0, DR):
            nc.vector.tensor_copy(rot[h:h + 32, :], kp[h + 32:h + 64, :])
            nc.vector.tensor_copy(rot[h + 32:h + 64, :], kp[h:h + 32, :])
        nc.vector.tensor_tensor(kp[:], kp[:], cos2p[:], MULT)
        nc.vector.tensor_tensor(rot[:], rot[:], sin2sp[:], MULT)
        nc.vector.tensor_tensor(kpo[:], kp[:], rot[:], ADD)

        # ship own latents; one big write each for fast queue drain
        nc.sync.dma_start(
            kv_out[:, 0:KPE_OFF], ckT[:].rearrange("p k w -> p (k w)"))
        nc.sync.dma_start(kv_out[:, KPE_OFF:KV_ELE], kpo[:])

    # gather the batch group's full kv set; emitted straight after the
    # kv_out writes so its (dominance-based) semaphore wait covers only
    # them — it fires while the q path computes
    nc.gpsimd.collective_compute(
        "AllGather", mybir.AluOpType.bypass,
        replica_groups=t["replica_groups"],
        ins=[kv_out.opt()], outs=[kv_gath.opt()])

    # phase D pools (opened after B's scratch is freed, before C's
    # scratch, so D's first DMAs carry no WAR hazard against C tiles)
    pdv = top.enter_context(tc.tile_pool(name="phD_v", bufs=1))
    pdk = top.enter_context(tc.tile_pool(name="phD_k", bufs=2))
    pew = top.enter_context(tc.tile_pool(name="phE_w", bufs=2))

    # ------------- phase A: qaT panel + rmsnorm ----------------------
    with tc.tile_pool(name="phA", bufs=2) as pa, \
         tc.tile_pool(name="psA", bufs=3, space="PSUM") as psA, \
         tc.tile_pool(name="psS", bufs=1, space="PSUM") as psSS, \
         tc.tile_pool(name="psB", bufs=1, space="PSUM") as psBC:
        ss = psSS.tile([1, W], F32, tag="ss")
        sq_prev = None
        for m in range(MB_QLR):
            wm = paw.tile([P, KB_HID, P], BF16, tag="wqa")
            nc.sync.dma_start(wm[:], t["w_qa"][m])
            ps = psA.tile([P, W], F32, tag="psA")
            for k in range(KB_HID):
                mm(ps[:], wm[:, k, :], hn0[:, k, :],
                   start=(k == 0), stop=(k == KB_HID - 1))
            nc.scalar.activation(qaT[:, m, :], ps[:], COPY)
            sq = pa.tile([P, W], BF16, tag="sq")
            nc.scalar.activation(sq[:], ps[:], SQUARE)
            if sq_prev is not None:
                mm(ss[:], ones_col[:], sq_prev,
                   start=(m == 1), stop=False, skip_group_check=True)
            sq_prev = sq[:]
        mm(ss[:], ones_col[:], sq_prev, start=False, stop=True,
           skip_group_check=True)
        rq = rsqrt_bcast(pa, psBC, ss[:], 1.0 / QLR)
        for m in range(MB_QLR):
            nc.vector.scalar_tensor_tensor(
                qaT[:, m, :], qaT[:, m, :], qa_ln[:, m:m + 1], rq[:],
                MULT, MULT)

    # ------------- phase C: q_b panel (+ RoPE on pe part) ------------
    with tc.tile_pool(name="phC", bufs=2) as pc, \
         tc.tile_pool(name="psA", bufs=3, space="PSUM") as psA:
        for m in range(MB_NOPE + MB_PE):
            wm = pcw.tile([P, KB_QLR, P], BF16, tag="wqb")
            nc.sync.dma_start(wm[:], t["w_qb"][m])
            ps = psA.tile([P, W], F32, tag="psA")
            for k in range(KB_QLR):
                mm(ps[:], wm[:, k, :], qaT[:, k, :],
                   start=(k == 0), stop=(k == KB_QLR - 1))
            if m < MB_NOPE:
                nc.scalar.activation(qnopeT[:, m, :], ps[:], COPY)
            else:
                j = m - MB_NOPE
                qb = pc.tile([P, W], BF16, tag="qb")
                nc.scalar.activation(qb[:], ps[:], COPY)
                rotq = pc.tile([P, W], BF16, tag="rotq")
                for h in (0, DR):
                    nc.vector.tensor_copy(rotq[h:h + 32, :],
                                          qb[h + 32:h + 64, :])
                    nc.vector.tensor_copy(rotq[h + 32:h + 64, :],
                                          qb[h:h + 32, :])
                nc.vector.tensor_tensor(rotq[:], rotq[:], sin2sp[:], MULT)
                nc.vector.tensor_tensor(qpeT[:, j, :], qb[:], cos2p[:], MULT)
                nc.vector.tensor_tensor(qpeT[:, j, :], qpeT[:, j, :],
                                        rotq[:], ADD)

    # ------------- phase D: attention over gathered keys -------------
    with tc.tile_pool(name="phD", bufs=2) as pd, \
         tc.tile_pool(name="probs", bufs=4) as pprob, \
         tc.tile_pool(name="psSc", bufs=4, space="PSUM") as psSc, \
         tc.tile_pool(name="psO", bufs=2, space="PSUM") as psO, \
         tc.tile_pool(name="psR", bufs=1, space="PSUM") as psR, \
         tc.tile_pool(name="psB2", bufs=1, space="PSUM") as psB2:
        dctx = ExitStack()
        mask_pool = None
        if with_mask:
            mask_pool = dctx.enter_context(tc.tile_pool(name="maskp", bufs=4))

        for r in range(NPANEL):
            nc.sync.dma_start(kpe2[:, r * W:(r + 1) * W],
                              kv_gath[r, :, KPE_OFF:KV_ELE])
            for kc in range(KB_CKV):
                nc.sync.dma_start(
                    ckTf[:, kc, r * W:(r + 1) * W],
                    kv_gath[r, :, kc * W:(kc + 1) * W])

        # deferred normalization finish of the previous head, emitted
        # late so its PE bcast / DVE reciprocal never stall the in-order
        # PE stream
        def finish_head(h, po, pr):
            rrow = pd.tile([1, W], F32, tag="rrow")
            nc.vector.reciprocal_approx_fast(rrow[:], pr[:])
            rrow_bf = pd.tile([1, W], BF16, tag="rrow_bf")
            nc.scalar.activation(rrow_bf[:], rrow[:], COPY)
            bc_ps = psB2.tile([P, W], F32, tag="bcd")
            mm(bc_ps[:], ones_row[:], rrow_bf[:], start=True, stop=True)
            bci = pd.tile([P, W], F32, tag="bci")
            nc.scalar.activation(bci[:], bc_ps[:], COPY)
            nc.vector.tensor_tensor(oT[:, h, :], po[:], bci[:], MULT)

        pending = None
        for g in range(NH // 2):
            # V for the 2 heads of this group: [128k, kb, 2*128]
            wv = pkw.tile([P, KB_CKV, 2 * DV], BF16, tag="wv")
            nc.sync.dma_start(wv[:], t["w_kvb_v"][g])
            v_sb = pdv.tile([P, KB_S, 2 * DV], BF16, tag="v")
            for kb in range(KB_S):
                psv = psSc.tile([P, W], F32, tag="pss")
                for kc in range(KB_CKV):
                    mm(psv[:, :2 * DV], ckTf[:, kc, kb * P:(kb + 1) * P],
                       wv[:, kc, :], start=(kc == 0), stop=(kc == KB_CKV - 1))
                nc.scalar.activation(v_sb[:, kb, :], psv[:, :2 * DV], COPY)

            for hl in range(2):
                h = g * 2 + hl
                wkn = pkw.tile([P, KB_CKV, DN], BF16, tag="wkn")
                nc.sync.dma_start(wkn[:], t["w_kvb_kn"][h])
                knT = pdk.tile([P, KB_S, P], BF16, tag="knT")
                for nch in range(NPANEL):
                    psk = psSc.tile([P, W], F32, tag="pss")
                    for kc in range(KB_CKV):
                        mm(psk[:], wkn[:, kc, :],
                           ckTf[:, kc, nch * W:(nch + 1) * W],
                           start=(kc == 0), stop=(kc == KB_CKV - 1))
                    nc.scalar.activation(
                        knT[:, nch * KB_W:(nch + 1) * KB_W, :],
                        psk[:], COPY)

                # attention for head h, software-pipelined over kb
                po = psO.tile([P, W], F32, tag="po")
                pr = psR.tile([1, W], F32, tag="pr")
                hp64 = hl * DR
                probs_q = []
                for kb in range(KB_S):
                    pss = psSc.tile([P, W], F32, tag="pss")
                    mm(pss[:], knT[:, kb, :], qnopeT[:, h, :],
                       start=True, stop=False)
                    mm(pss[:], kpe2[hp64:hp64 + DR, kb * P:(kb + 1) * P],
                       qpeT[hp64:hp64 + DR, g, :], start=False, stop=True)
                    probs = pprob.tile([P, W], BF16, tag="probs")
                    if with_mask:
                        mtile = mask_pool.tile([P, W], F32, tag="mt")
                        nc.sync.dma_start(
                            mtile[:], t["maskT"][kb * P:(kb + 1) * P, :])
                        pf = pprob.tile([P, W], F32, tag="probs_f")
                        nc.vector.scalar_tensor_tensor(
                            pf[:], pss[:], SCALE, mtile[:], MULT, ADD)
                        nc.scalar.activation(probs[:], pf[:], EXP)
                    else:
                        nc.scalar.activation(probs[:], pss[:], EXP,
                                             scale=SCALE)
                    probs_q.append((kb, probs))
                    if len(probs_q) == 4 or kb == KB_S - 1:
                        for kb2, pb2 in probs_q:
                            mm(po[:], v_sb[:, kb2, hl * DV:(hl + 1) * DV],
                               pb2[:], start=(kb2 == 0),
                               stop=(kb2 == KB_S - 1), skip_group_check=True)
                            mm(pr[:], ones_col[:], pb2[:],
                               start=(kb2 == 0), stop=(kb2 == KB_S - 1),
                               skip_group_check=True)
                        probs_q = []
                        # finish the previous head once this head's first
                        # kb batch is in flight: its reciprocal has had
                        # time to drain, so the PE bcast never stalls
                        if pending is not None:
                            finish_head(*pending)
                            pending = None
                pending = (h, po, pr)
        finish_head(*pending)
        dctx.close()

    # ------------- phase E: o_proj -----------------------------------
    with tc.tile_pool(name="phE", bufs=2) as pe, \
         tc.tile_pool(name="psA", bufs=3, space="PSUM") as psA:
        for m in range(MB_HID):
            wm = pew.tile([P, NH, P], BF16, tag="wo")
            nc.sync.dma_start(wm[:], t["w_o"][m])
            ps = psA.tile([P, W], F32, tag="psA")
            for k in range(NH):
                mm(ps[:], wm[:, k, :], oT[:, k, :],
                   start=(k == 0), stop=(k == NH - 1))
            osb = pe.tile([P, W], F32, tag="osb")
            nc.scalar.activation(osb[:], ps[:], COPY)
            nc.sync.dma_start(t["outT"][m * P:(m + 1) * P, :], osb[:])
    top.close()


def _build_program(with_mask):
    nc = bacc.Bacc("TRN2", target_bir_lowering=False, debug=False)
    t = {"replica_groups": [list(range(b * NPANEL, (b + 1) * NPANEL))
                            for b in range(B)]}

    def inp(name, shape, dt=BF16):
        t[name] = nc.dram_tensor(name, list(shape), dt,
                                 kind="ExternalInput").ap()

    inp("hs_pkp", [P, KB_HID, W])
    inp("w_qa", [MB_QLR, P, KB_HID, P])
    inp("w_qb", [MB_NOPE + MB_PE, P, KB_QLR, P])
    inp("w_kva", [P, MB_KVA, KB_HID, P])
    inp("w_kvb_kn", [NH, P, KB_CKV, DN])
    inp("w_kvb_v", [NH // 2, P, KB_CKV, 2 * DV])
    inp("w_o", [MB_HID, P, NH, P])
    inp("qa_ln_p", [P, MB_QLR], F32)
    inp("kva_ln_p", [P, KB_CKV], F32)
    inp("cos2p", [P, W])
    inp("sin2sp", [P, W])
    if with_mask:
        inp("maskT", [S, W], F32)
    t["outT"] = nc.dram_tensor("outT", [HID, W], F32,
                               kind="ExternalOutput").ap()

    with tile.TileContext(nc) as tc:
        _emit(tc, t, with_mask)
    nc.compile()
    return nc


_PROG_CACHE = {}


def _get_program(with_mask):
    if with_mask not in _PROG_CACHE:
        _PROG_CACHE[with_mask] = _build_program(with_mask)
    return _PROG_CACHE[with_mask]


def _block4(w, mb, kb):
    """[kb*P, mb*P] -> [mb, P, kb, P] with W[m,p,k,c] = w[k*P+p, m*P+c]."""
    return np.ascontiguousarray(
        w.reshape(kb, P, mb, P).transpose(2, 1, 0, 3))


def make_in_maps(hidden_states, attention_mask, cos, sin, w_qa, qa_ln, w_qb,
                 w_kva, kva_ln, w_kvb, w_o, with_mask):
    f32 = np.float32
    c = np.ascontiguousarray

    w_qb_r = np.asarray(w_qb, f32).reshape(QLR, NH, DQK)
    w_qb_re = np.concatenate(
        [w_qb_r[:, :, :DN].reshape(QLR, NH * DN),
         w_qb_r[:, :, DN:].reshape(QLR, NH * DR)], axis=1)
    w_kva_pad = np.concatenate(
        [np.asarray(w_kva, f32), np.zeros((HID, P - DR), f32)], axis=1)
    kvb = np.asarray(w_kvb, f32).reshape(KB_CKV, P, NH, DN + DV)
    w_kvb_kn = c(kvb[:, :, :, :DN].transpose(2, 1, 0, 3)
                 .astype(NPBF))                        # [NH, P, KB_CKV, DN]
    w_kvb_v = c(kvb[:, :, :, DN:].reshape(KB_CKV, P, NH // 2, 2 * DV)
                .transpose(2, 1, 0, 3).astype(NPBF))   # [NH/2, P, kc, 256]

    qa_ln_p = c(np.asarray(qa_ln, f32).reshape(MB_QLR, P).T)
    kva_ln_p = c(np.asarray(kva_ln, f32).reshape(KB_CKV, P).T)

    cosT = np.asarray(cos, f32).T                      # [64, S]
    sinT = np.asarray(sin, f32).T
    sin_s = np.concatenate([-sinT[:DR // 2], sinT[DR // 2:]], axis=0)
    cos2 = c(np.concatenate([cosT, cosT], axis=0))     # [128, S]
    sin2s = c(np.concatenate([sin_s, sin_s], axis=0))

    shared = {
        "w_qa": _block4(np.asarray(w_qa, f32), MB_QLR, KB_HID).astype(NPBF),
        "w_qb": _block4(w_qb_re, MB_NOPE + MB_PE, KB_QLR).astype(NPBF),
        "w_kva": c(w_kva_pad.reshape(KB_HID, P, MB_KVA, P)
                   .transpose(1, 2, 0, 3).astype(NPBF)),
        "w_kvb_kn": w_kvb_kn,
        "w_kvb_v": w_kvb_v,
        "w_o": _block4(np.asarray(w_o, f32), MB_HID, KB_HID).astype(NPBF),
        "qa_ln_p": qa_ln_p,
        "kva_ln_p": kva_ln_p,
    }

    hs = np.asarray(hidden_states)
    am = np.asarray(attention_mask)
    in_maps = []
    for core in range(NCORES):
        b, pnl = divmod(core, NPANEL)
        q0 = pnl * W
        hsT = np.asarray(hs[b], f32).T[:, q0:q0 + W]   # [HID, W]
        m = dict(shared)
        m["hs_pkp"] = c(hsT.reshape(KB_HID, P, W).transpose(1, 0, 2)
                        .astype(NPBF))                 # [128, 16, W]
        m["cos2p"] = c(cos2[:, q0:q0 + W].astype(NPBF))
        m["sin2sp"] = c(sin2s[:, q0:q0 + W].astype(NPBF))
        if with_mask:
            m["maskT"] = c(am[b, 0, q0:q0 + W, :].T.astype(f32))
        in_maps.append(m)
    return in_maps


def kernel(hidden_states, attention_mask, cos, sin, w_qa, qa_ln, w_qb,
           w_kva, kva_ln, w_kvb, w_o):
    global LAST_RESULT
    with_mask = bool(np.any(np.asarray(attention_mask) != 0))
    nc = _get_program(with_mask)
    in_maps = make_in_maps(hidden_states, attention_mask, cos, sin, w_qa,
                           qa_ln, w_qb, w_kva, kva_ln, w_kvb, w_o, with_mask)
    trace = os.environ.get("KERNEL_TRACE", "0") == "1"
    res = bass_utils.run_bass_kernel_spmd(
        nc, in_maps, core_ids=list(range(NCORES)), trace=trace)
    LAST_RESULT = res

    out = np.empty((B, S, HID), np.float32)
    for core in range(NCORES):
        b, pnl = divmod(core, NPANEL)
        q0 = pnl * W
        out[b, q0:q0 + W, :] = res.results[core]["outT"].T
    return out
